# revision 1
# baseline (speedup 1.0000x reference)
import numpy as np
import jax
import jax.numpy as jnp
from functools import partial

# nn_LinearConvAttention: B=2, DIM=256, H=W=D=48, 4 heads,
# head_dim_qk=32, head_dim_v=64. Sharding: 8 cores = (b, head) pairs,
# fully independent (no communication), per the tensor-parallel head split.
B = 2
DIM = 256
HGRID = 48
NH = 4
DQK = 32
DV = 64
N = HGRID * HGRID * HGRID
EPS = 1e-6


@partial(jax.jit, static_argnums=())
def _shard_compute(x_b, wq_h, wk_h, wv_h, bv_h):
    """One (batch, head) shard on one core.

    x_b   [256, 48, 48, 48] : full-channel input for this batch element
    wq_h  [32, 256], wk_h [32, 256]
    wv_h  [64, 3, 3, 3]     : depthwise taps for this head's 64 v-channels
    bv_h  [64]
    returns out_h [64, N]
    """
    xf = x_b.reshape(DIM, N)

    # 1x1x1 convs == channel matmuls
    q = wq_h @ xf                          # [32, N]
    k = wk_h @ xf                          # [32, N]

    # depthwise 3x3x3 conv, padding 1, on this head's 64 channels
    xh = x_b[:, :, :, :]                   # [256,48,48,48]
    xv = jax.lax.dynamic_slice_in_dim(xh, 0, DIM, 0)  # no-op keep full
    # channels for v of this head are selected on host; here x_b_v passed via wv trick
    # (we pass the pre-sliced 64 channels as the last 64 rows? -> instead host slices)
    return q, k  # placeholder (replaced below)


def _make_shard_fn():
    def f(x_b, x_v, wq_h, wk_h, wv_h, bv_h):
        # x_b  [256, 48,48,48]  full channels (for q/k projections)
        # x_v  [64, 48,48,48]   this head's v-channel slice of x_b
        xf = x_b.reshape(DIM, N)
        q = wq_h @ xf                      # [32, N]
        k = wk_h @ xf                      # [32, N]

        xp = jnp.pad(x_v, ((0, 0), (1, 1), (1, 1), (1, 1)))
        v = jnp.zeros((DV, HGRID, HGRID, HGRID), dtype=x_v.dtype)
        for i in range(3):
            for j in range(3):
                for kk in range(3):
                    w = wv_h[:, i, j, kk][:, None, None, None]
                    v = v + w * jax.lax.slice(
                        xp, (0, i, j, kk), (DV, i + HGRID, j + HGRID, kk + HGRID)
                    )
        v = (v + bv_h[:, None, None, None]).reshape(DV, N)

        q = jax.nn.softmax(q, axis=0)      # over per-head channel dim
        k = jax.nn.softmax(k, axis=1)      # over spatial dim

        kv = k @ v.T                       # [32, 64]
        ksum = k.sum(axis=1)               # [32]
        num = kv.T @ q                     # [64, N]
        Z = ksum @ q                       # [N]
        out = num / (Z[None, :] + EPS)     # [64, N]
        return out

    return jax.jit(f)


_SHARD_FN = None


def kernel(x, Wq, Wk, Wv, bv):
    """Full inputs in, full output out. Shards (b, head) across 8 cores."""
    global _SHARD_FN
    devices = jax.devices()
    assert len(devices) >= 8, devices
    if _SHARD_FN is None:
        _SHARD_FN = _make_shard_fn()

    x = np.asarray(x, dtype=np.float32)
    Wq = np.asarray(Wq, dtype=np.float32)
    Wk = np.asarray(Wk, dtype=np.float32)
    Wv = np.asarray(Wv, dtype=np.float32)
    bv = np.asarray(bv, dtype=np.float32)

    outs = []
    futures = []
    for core in range(8):
        b, h = divmod(core, NH)
        dev = devices[core]
        x_b = jax.device_put(x[b], dev)
        x_v = jax.device_put(x[b, h * DV:(h + 1) * DV], dev)
        wq_h = jax.device_put(Wq[h * DQK:(h + 1) * DQK], dev)
        wk_h = jax.device_put(Wk[h * DQK:(h + 1) * DQK], dev)
        wv_h = jax.device_put(Wv[h * DV:(h + 1) * DV, 0], dev)
        bv_h = jax.device_put(bv[h * DV:(h + 1) * DV], dev)
        futures.append((core, _SHARD_FN(x_b, x_v, wq_h, wk_h, wv_h, bv_h)))

    out = np.empty((B, DIM, N), dtype=np.float32)
    for core, fut in futures:
        b, h = divmod(core, NH)
        out[b, h * DV:(h + 1) * DV] = np.asarray(fut)
    return out.reshape(B, DIM, HGRID, HGRID, HGRID)



# revision 6
# speedup vs baseline: 3.6964x; 3.6964x over previous
"""nn_LinearConvAttention Trainium2 Bass kernel.

B=2, C=256, 48^3 grid, 4 heads (dqk=32, dv=64). 8 NeuronCores.

Sharding: 8-way spatial over H. Core c computes output planes [6c, 6c+6) of
both batch elements, all 256 channels. Inputs are uploaded as bf16 shards
[2, 256, 8, 48*48] (6 interior planes + 1 halo plane each side, zero-padded at
the global boundary). The only cross-core communication is a 131KB AllReduce
of the per-head kv/ksum statistics (kv contracts over the full spatial axis).

Math (per batch b):
  q = Wq x ; k = Wk x ; v = dwconv3x3x3(x) + bv
  ke = exp(k)                      (k rows are O(0.3); exp safe unshifted)
  kv[r, c] = sum_n ke[r, n] v[c, n]   ;  S[r] = sum_n ke[r, n]   (AllReduce)
  A = blockdiag_mask * kv / S[:, None]
  eq = exp(q)
  y[c, n] = sum_r A[r, c] eq[r, n]
  D[h, n] = sum_{r in h} eq[r, n]
  out = y / (D * (1 + eps))        (division folded into the host upcast;
                                    the reference's Z term == 1 exactly)
"""

import atexit
import contextlib
from concurrent.futures import ThreadPoolExecutor
from dataclasses import dataclass

import numpy as np
import ml_dtypes

import concourse.bacc as bacc
import concourse.mybir as mybir
from concourse.tile import TileContext

BF16 = mybir.dt.bfloat16
F32 = mybir.dt.float32
I32 = mybir.dt.int32
EPS = 1e-6


@dataclass
class Cfg:
    B: int = 2
    C: int = 256
    NH: int = 4
    DQK: int = 32
    DV: int = 64
    W: int = 48
    D: int = 48
    PP: int = 6           # output planes per core
    NCORES: int = 8
    pe_ct: tuple = (True, False)   # conv unit engine per ctile: PE / DVE
    qk_chunk: int = 384
    num_chunk: int = 512

    @property
    def WD(self):
        return self.W * self.D

    @property
    def NC(self):
        return self.PP * self.WD

    @property
    def PIN(self):
        return self.PP + 2

    @property
    def DP(self):
        return self.D + 4   # padded D pitch (interior at col offset 2)

    @property
    def WDP(self):
        return (self.W + 2) * self.DP


def _tapidx(di, dj, dk):
    return (di + 1) * 9 + (dj + 1) * 3 + (dk + 1)


def build_nc(cfg: Cfg):
    WD, PIN, NC = cfg.WD, cfg.PIN, cfg.NC
    assert WD % 128 == 0

    nc = bacc.Bacc("TRN2", target_bir_lowering=False, debug=False,
                   num_devices=cfg.NCORES)

    x_in = nc.dram_tensor("x", [cfg.B, cfg.C, PIN, WD], BF16,
                          kind="ExternalInput").ap()
    wqk = nc.dram_tensor("wqk", [2, 128, 256], BF16, kind="ExternalInput").ap()
    wv = nc.dram_tensor("wv", [2, 27, 128], F32, kind="ExternalInput").ap()
    bv = nc.dram_tensor("bv", [2, 128], F32, kind="ExternalInput").ap()
    basis4 = nc.dram_tensor("basis4", [128, cfg.NH], BF16,
                            kind="ExternalInput").ap()
    bmask = nc.dram_tensor("bmask", [128, 256], BF16,
                           kind="ExternalInput").ap()
    y_out = nc.dram_tensor("y", [cfg.B, cfg.C, NC], BF16,
                           kind="ExternalOutput").ap()
    d_out = nc.dram_tensor("dnm", [cfg.B, cfg.NH, NC], F32,
                           kind="ExternalOutput").ap()
    cc_in = nc.dram_tensor("cc_in", [cfg.B, 128, 257], F32)
    cc_out = nc.dram_tensor("cc_out", [cfg.B, 128, 257], F32)

    with TileContext(nc) as tc:
        _emit(nc, tc, cfg, x_in, wqk, wv, bv, basis4, bmask, y_out, d_out,
              cc_in, cc_out)
    nc.compile()
    return nc


def _clip(s, n):
    """shift s in {-1,0,1}: returns (out_start, in_start, count)."""
    if s < 0:
        return 1, 0, n - 1
    if s > 0:
        return 0, 1, n - 1
    return 0, 0, n


def _emit(nc, tc, cfg, x_in, wqk, wv, bv, basis4, bmask, y_out, d_out,
          cc_in, cc_out):
    WD, PP, PIN, NC, W, D = cfg.WD, cfg.PP, cfg.PIN, cfg.NC, cfg.W, cfg.D
    NCH = WD // 128
    QKC = cfg.qk_chunk
    assert WD % QKC == 0
    NQK = WD // QKC
    NMC = cfg.num_chunk
    assert NC % NMC == 0
    NNM = NC // NMC
    TAPS = [(di, dj, dk) for di in (-1, 0, 1) for dj in (-1, 0, 1)
            for dk in (-1, 0, 1)]
    TAPS.remove((0, 0, 0))
    TAPS.insert(0, (0, 0, 0))  # center first: defines psum/acc init

    ctx = contextlib.ExitStack()
    with ctx:
        const_p = ctx.enter_context(tc.tile_pool(name="const", bufs=1))
        xdv_p = ctx.enter_context(tc.tile_pool(name="xdv", bufs=4))
        xod_p = ctx.enter_context(tc.tile_pool(name="xod", bufs=4))
        eq_p = ctx.enter_context(tc.tile_pool(name="eq", bufs=2))
        ke_p = ctx.enter_context(tc.tile_pool(name="ke", bufs=2))
        v_p = ctx.enter_context(tc.tile_pool(name="v", bufs=3))
        keT_p = ctx.enter_context(tc.tile_pool(name="keT", bufs=2))
        vT_p = ctx.enter_context(tc.tile_pool(name="vT", bufs=2))
        out_p = ctx.enter_context(tc.tile_pool(name="out", bufs=3))
        small_p = ctx.enter_context(tc.tile_pool(name="small", bufs=2))
        akv_p = ctx.enter_context(tc.tile_pool(name="akv", bufs=2))

        qk_ps = ctx.enter_context(tc.tile_pool(name="qkps", bufs=1, space="PSUM"))
        cv_ps = ctx.enter_context(tc.tile_pool(name="cvps", bufs=1, space="PSUM"))
        kv_ps = ctx.enter_context(tc.tile_pool(name="kvps", bufs=1, space="PSUM"))
        nm_ps = ctx.enter_context(tc.tile_pool(name="nmps", bufs=1, space="PSUM"))

        # ---- constants ----
        wqk_sb = const_p.tile([128, 2, 256], BF16, tag="wqk")
        nc.sync.dma_start(out=wqk_sb[:, :, :],
                          in_=wqk.rearrange("t c m -> c t m"))
        wv_sb = const_p.tile([128, 2, 27], F32, tag="wv")
        nc.sync.dma_start(out=wv_sb[:, :, :],
                          in_=wv.rearrange("t k c -> c t k"))
        bv_sb = const_p.tile([128, 2], F32, tag="bv")
        nc.sync.dma_start(out=bv_sb[:, :], in_=bv.rearrange("t c -> c t"))
        basis_sb = const_p.tile([128, cfg.NH], BF16, tag="basis")
        nc.sync.dma_start(out=basis_sb[:, :], in_=basis4[:, :])
        mask_sb = const_p.tile([128, 256], BF16, tag="bmask")
        nc.sync.dma_start(out=mask_sb[:, :], in_=bmask[:, :])
        ones_sb = const_p.tile([128, 1], BF16, tag="ones")
        nc.vector.memset(ones_sb[:, :], 1.0)

        # identity & per-tap diagonal weight matrices for the PE conv ctiles
        iot = const_p.tile([128, 128], I32, tag="iot")
        nc.gpsimd.iota(iot[:, :], pattern=[[1, 128]], base=0,
                       channel_multiplier=-1)
        ident = const_p.tile([128, 128], BF16, tag="ident")
        nc.vector.tensor_scalar(ident[:, :], iot[:, :], 0, None,
                                op0=mybir.AluOpType.is_equal)
        diags = {}
        for ct in range(2):
            if not cfg.pe_ct[ct]:
                continue
            dg = const_p.tile([128, 27, 128], BF16, tag=f"diag{ct}")
            for t in range(27):
                nc.vector.tensor_scalar(dg[:, t, :], ident[:, :],
                                        wv_sb[:, ct, t:t + 1], None,
                                        op0=mybir.AluOpType.mult)
            diags[ct] = dg

        # persistent padded-x ring for the PE conv ctiles (borders stay 0)
        DP, WDP = cfg.DP, cfg.WDP
        XPE_SLOTS = 5
        xpe_ring = {}
        for ct in range(2):
            if not cfg.pe_ct[ct]:
                continue
            for s in range(XPE_SLOTS):
                t = const_p.tile([128, WDP], BF16, tag=f"xpr{ct}_{s}")
                nc.vector.memset(t[:, :], 0.0)
                xpe_ring[(ct, s)] = t
        ring_ctr = {ct: 0 for ct in range(2)}

        kv_sb = {}
        eq_sl = {}

        # =============== main loop over batches ===============
        for b in range(cfg.B):
            eq_slab = eq_p.tile([128, NC], BF16, tag="eq")
            eq_sl[b] = eq_slab
            kv_tile = kv_ps.tile([128, 256], F32, tag="kv")
            kvS_tile = kv_ps.tile([128, 1], F32, tag="kvS")
            first_kv = [True]

            xs = {}
            xso = {}

            def load_plane(pl, b=b, xs=xs, xso=xso):
                for ct in range(2):
                    if (pl, ct) in xs:
                        continue
                    src_ap = x_in[b, ct * 128:(ct + 1) * 128, pl, :]
                    if cfg.pe_ct[ct]:
                        t = xpe_ring[(ct, ring_ctr[ct] % XPE_SLOTS)]
                        ring_ctr[ct] += 1
                        dst = t[:, :].rearrange("c (w d) -> c w d", d=DP)
                        nc.sync.dma_start(
                            out=dst[:, 1:W + 1, 2:D + 2],
                            in_=src_ap.rearrange("c (w d) -> c w d", d=D))
                        xs[(pl, ct)] = t
                    else:
                        t = xdv_p.tile([128, WD], BF16, tag=f"x{ct}")
                        nc.sync.dma_start(out=t[:, :], in_=src_ap)
                        xs[(pl, ct)] = t
                        to = xod_p.tile([128, WD], BF16, tag=f"xo{ct}")
                        # to[:, j] = x[:, j+1]; last element garbage, never read
                        nc.scalar.dma_start(
                            out=to[:, 0:WD - 1],
                            in_=x_in[b, ct * 128:(ct + 1) * 128, pl, 1:WD])
                        xso[(pl, ct)] = to

            for pl in range(min(3, PIN)):
                load_plane(pl)

            for p in range(PP):
                if p + 3 < PIN:
                    load_plane(p + 3)

                # ---- q/k projections (input plane p+1) + exp ----
                ke_pl = ke_p.tile([128, WD], BF16, tag="ke")
                for ch in range(NQK):
                    qp = qk_ps.tile([128, QKC], F32, tag="qps")
                    kp = qk_ps.tile([128, QKC], F32, tag="kps")
                    rows_per_qk = QKC // D
                    for ct in range(2):
                        if cfg.pe_ct[ct]:
                            xv = xs[(p + 1, ct)][:, :].rearrange(
                                "c (w d) -> c w d", d=DP)
                            r0 = ch * rows_per_qk
                            rhs = xv[:, 1 + r0:1 + r0 + rows_per_qk, 2:D + 2]
                        else:
                            rhs = xs[(p + 1, ct)][:, ch * QKC:(ch + 1) * QKC]
                        nc.tensor.matmul(qp[:, :], wqk_sb[:, ct, 0:128], rhs,
                                         start=(ct == 0), stop=(ct == 1))
                        nc.tensor.matmul(kp[:, :], wqk_sb[:, ct, 128:256], rhs,
                                         start=(ct == 0), stop=(ct == 1))
                    nc.scalar.activation(
                        eq_slab[:, p * WD + ch * QKC:p * WD + (ch + 1) * QKC],
                        qp[:, :], mybir.ActivationFunctionType.Exp)
                    nc.scalar.activation(ke_pl[:, ch * QKC:(ch + 1) * QKC],
                                         kp[:, :],
                                         mybir.ActivationFunctionType.Exp)

                # ---- depthwise conv ----
                v_pl = {}
                for ct in range(2):
                    vt = v_p.tile([128, WD], BF16, tag=f"v{ct}")
                    v_pl[ct] = vt
                    if cfg.pe_ct[ct]:
                        _conv_pe(nc, cfg, cv_ps, vt, xs, diags[ct], bv_sb,
                                 ct, p, TAPS)
                    else:
                        _conv_dve(nc, cfg, vt, xs, xso, wv_sb, bv_sb,
                                  ct, p, TAPS)

                # ---- transposes + kv accumulation ----
                keT = keT_p.tile([128, NCH, 128], BF16, tag="keT")
                nc.sync.dma_start_transpose(keT[:, :, :], ke_pl[:, :])
                vT = vT_p.tile([128, NCH, 256], BF16, tag="vT")
                nc.sync.dma_start_transpose(vT[:, :, 0:128], v_pl[0][:, :])
                nc.scalar.dma_start_transpose(vT[:, :, 128:256], v_pl[1][:, :])
                for ch in range(NCH):
                    st = first_kv[0]
                    last = (p == PP - 1 and ch == NCH - 1)
                    nc.tensor.matmul(kv_tile[:, :], keT[:, ch, :],
                                     vT[:, ch, :], start=st, stop=last,
                                     skip_group_check=True)
                    nc.tensor.matmul(kvS_tile[:, :], keT[:, ch, :],
                                     ones_sb[:, :], start=st, stop=last,
                                     skip_group_check=True)
                    first_kv[0] = False

            # ---- AllReduce kv / S, then A = mask * kv / S ----
            kvs = akv_p.tile([128, 257], F32, tag="kvsb")
            nc.vector.tensor_copy(kvs[:, 0:256], kv_tile[:, :])
            nc.vector.tensor_copy(kvs[:, 256:257], kvS_tile[:, :])
            nc.sync.dma_start(out=cc_in.ap()[b, :, :], in_=kvs[:, :])
            nc.gpsimd.collective_compute(
                "AllReduce", mybir.AluOpType.add,
                replica_groups=[list(range(cfg.NCORES))],
                ins=[cc_in.ap()[b:b + 1, :, :].opt()],
                outs=[cc_out.ap()[b:b + 1, :, :].opt()])
            kvr = akv_p.tile([128, 257], F32, tag="kvr")
            nc.scalar.dma_start(out=kvr[:, :], in_=cc_out.ap()[b, :, :])
            rS = small_p.tile([128, 1], F32, tag="rS")
            nc.vector.reciprocal(rS[:, :], kvr[:, 256:257])
            A_sb = akv_p.tile([128, 256], BF16, tag="A")
            nc.vector.scalar_tensor_tensor(
                A_sb[:, :], kvr[:, 0:256], rS[:, 0:1], mask_sb[:, :],
                op0=mybir.AluOpType.mult, op1=mybir.AluOpType.mult)
            kv_sb[b] = A_sb

        # =============== output phase ===============
        for b in range(cfg.B):
            A_sb = kv_sb[b]
            eq_slab = eq_sl[b]
            for ch in range(NNM):
                rhs = eq_slab[:, ch * NMC:(ch + 1) * NMC]
                n0 = nm_ps.tile([128, NMC], F32, tag="n0")
                n1 = nm_ps.tile([128, NMC], F32, tag="n1")
                dr = nm_ps.tile([cfg.NH, NMC], F32, tag="dr")
                nc.tensor.matmul(n0[:, :], A_sb[:, 0:128], rhs,
                                 start=True, stop=True)
                nc.tensor.matmul(n1[:, :], A_sb[:, 128:256], rhs,
                                 start=True, stop=True)
                nc.tensor.matmul(dr[:, :], basis_sb[:, :], rhs,
                                 start=True, stop=True)
                o0 = out_p.tile([128, NMC], BF16, tag="o0")
                o1 = out_p.tile([128, NMC], BF16, tag="o1")
                nc.vector.tensor_copy(o0[:, :], n0[:, :])
                nc.scalar.copy(o1[:, :], n1[:, :])
                nc.sync.dma_start(
                    out=y_out[b, 0:128, ch * NMC:(ch + 1) * NMC], in_=o0[:, :])
                nc.sync.dma_start(
                    out=y_out[b, 128:256, ch * NMC:(ch + 1) * NMC],
                    in_=o1[:, :])
                od = out_p.tile([cfg.NH, NMC], F32, tag="od")
                nc.vector.tensor_copy(od[:, :], dr[:, :])
                nc.gpsimd.dma_start(
                    out=d_out[b, :, ch * NMC:(ch + 1) * NMC], in_=od[:, :])


def _conv_pe(nc, cfg, cv_ps, vt, xs, dg, bv_sb, ct, p, taps):
    """Conv unit on PE: per-tap diagonal-weight matmuls accumulating into
    PSUM pieces of the plane; input is the zero-padded plane tile so every
    tap is a uniform full-width window. Evicted via ACT with +bv bias."""
    W, D, DP = cfg.W, cfg.D, cfg.DP
    rows_per = max(1, 512 // D)
    n_pieces = (W + rows_per - 1) // rows_per
    for pc in range(n_pieces):
        t0, t1 = pc * rows_per, min(W, (pc + 1) * rows_per)
        nr = t1 - t0
        ps = cv_ps.tile([128, nr * D], F32, tag="cv")
        for i, (di, dj, dk) in enumerate(taps):
            xv = xs[(p + 1 + di, ct)][:, :].rearrange(
                "c (w d) -> c w d", d=DP)
            rhs = xv[:, t0 + dj + 1:t1 + dj + 1, 2 + dk:2 + dk + D]
            nc.tensor.matmul(
                ps[:, :], dg[:, _tapidx(di, dj, dk), :], rhs,
                start=(i == 0), stop=(i == len(taps) - 1),
                skip_group_check=True)
        nc.scalar.activation(
            vt[:, t0 * D:t1 * D], ps[:, :],
            mybir.ActivationFunctionType.Identity,
            bias=bv_sb[:, ct:ct + 1])


def _conv_dve(nc, cfg, vt, xs, xso, wv_sb, bv_sb, ct, p, taps):
    """Conv unit on DVE: scalar_tensor_tensor FMA into the bf16 v tile.
    D-axis (innermost) alignment for 2x mode:
      dk=0  : both APs 4B-aligned as-is
      dk=+1 : src from the odd-shifted copy xo (xo[:, j] = x[:, j+1])
      dk=-1 : dst cols [2, D) with src xo cols [0, D-2); col 1 fixed up
              with a small strided op (col 0 needs no contribution).
    """
    W, D = cfg.W, cfg.D

    def w_ap(tap):
        i = _tapidx(*tap)
        return wv_sb[:, ct, i:i + 1]

    for i, (di, dj, dk) in enumerate(taps):
        ow0, iw0, wcnt = _clip(dj, W)
        xt = xs[(p + 1 + di, ct)]
        ov = vt[:, :].rearrange("c (w d) -> c w d", d=D)
        if i == 0:
            nc.vector.tensor_scalar(
                vt[:, :], xt[:, :], w_ap((0, 0, 0)), bv_sb[:, ct:ct + 1],
                op0=mybir.AluOpType.mult, op1=mybir.AluOpType.add)
            continue
        if dk == 0:
            xv = xt[:, :].rearrange("c (w d) -> c w d", d=D)
            dst = ov[:, ow0:ow0 + wcnt, :]
            src = xv[:, iw0:iw0 + wcnt, :]
        elif dk == 1:
            xo = xso[(p + 1 + di, ct)][:, :].rearrange("c (w d) -> c w d", d=D)
            dst = ov[:, ow0:ow0 + wcnt, 0:D - 1]
            src = xo[:, iw0:iw0 + wcnt, 0:D - 1]
        else:  # dk == -1
            xo = xso[(p + 1 + di, ct)][:, :].rearrange("c (w d) -> c w d", d=D)
            dst = ov[:, ow0:ow0 + wcnt, 2:D]
            src = xo[:, iw0:iw0 + wcnt, 0:D - 2]
        nc.vector.scalar_tensor_tensor(
            dst, src, w_ap((di, dj, dk)), dst,
            op0=mybir.AluOpType.mult, op1=mybir.AluOpType.add)
        if dk == -1:
            xv = xt[:, :].rearrange("c (w d) -> c w d", d=D)
            d1 = ov[:, ow0:ow0 + wcnt, 1:2]
            s0 = xv[:, iw0:iw0 + wcnt, 0:1]
            nc.vector.scalar_tensor_tensor(
                d1, s0, w_ap((di, dj, dk)), d1,
                op0=mybir.AluOpType.mult, op1=mybir.AluOpType.add)


# ======================================================================
# host side
# ======================================================================

_STATE = {}
_POOL = ThreadPoolExecutor(16)


def _mt_copy(dst, src):
    n = dst.shape[0]
    cs = max(1, (n + 15) // 16)

    def work(i):
        dst[i * cs:(i + 1) * cs] = src[i * cs:(i + 1) * cs]
    list(_POOL.map(work, range((n + cs - 1) // cs)))


def prep_inputs(x, Wq, Wk, Wv27, bvec, cfg: Cfg):
    B, C, PP, PIN, WD = cfg.B, cfg.C, cfg.PP, cfg.PIN, cfg.WD
    NCORES = cfg.NCORES
    HH = NCORES * PP

    xr = np.ascontiguousarray(x.reshape(B, C, HH, WD))
    xpad = np.zeros((B, C, HH + 2, WD), ml_dtypes.bfloat16)
    _mt_copy(xpad.reshape(B * C, HH + 2, WD)[:, 1:HH + 1],
             xr.reshape(B * C, HH, WD))
    s = xpad.strides
    Gv = np.lib.stride_tricks.as_strided(
        xpad, (NCORES, B, C, PIN, WD), (PP * s[2], s[0], s[1], s[2], s[3]))
    G = np.empty((NCORES, B, C, PIN, WD), ml_dtypes.bfloat16)
    _mt_copy(G, Gv)
    x_global = G.reshape(NCORES * B, C, PIN, WD)

    wqk1 = np.empty((2, 128, 256), ml_dtypes.bfloat16)
    for ct in range(2):
        wqk1[ct, :, 0:128] = Wq[:, ct * 128:(ct + 1) * 128].T
        wqk1[ct, :, 128:256] = Wk[:, ct * 128:(ct + 1) * 128].T
    wv1 = np.empty((2, 27, 128), np.float32)
    for ct in range(2):
        wv1[ct] = Wv27[ct * 128:(ct + 1) * 128].T
    bv1 = bvec.reshape(2, 128).astype(np.float32)
    basis = np.zeros((128, cfg.NH), ml_dtypes.bfloat16)
    for r in range(128):
        basis[r, r // cfg.DQK] = 1
    bm = np.zeros((128, 256), ml_dtypes.bfloat16)
    for r in range(128):
        h = r // cfg.DQK
        bm[r, h * cfg.DV:(h + 1) * cfg.DV] = 1
    rep = lambda a: np.broadcast_to(
        a[None], (NCORES,) + a.shape).reshape((NCORES * a.shape[0],) +
                                              a.shape[1:])
    return {
        "x": x_global,
        "wqk": np.ascontiguousarray(rep(wqk1)),
        "wv": np.ascontiguousarray(rep(wv1)),
        "bv": np.ascontiguousarray(rep(bv1)),
        "basis4": np.ascontiguousarray(rep(basis)),
        "bmask": np.ascontiguousarray(rep(bm)),
    }


def build_runner(nc, cfg: Cfg):
    """One shard_map'd jit over 8 cores; donated output buffers are created
    on-device (the stock run_bass_via_pjrt uploads host zeros every call)."""
    import jax
    import jax.numpy as jnp
    from jax.experimental.shard_map import shard_map
    from jax.sharding import Mesh, PartitionSpec, NamedSharding
    from concourse import bass2jax

    bass2jax.install_neuronx_cc_hook()

    partition_name = (nc.partition_id_tensor.name
                      if nc.partition_id_tensor else None)
    in_names, out_names, out_avals = [], [], []
    for alloc in nc.m.functions[0].allocations:
        if not isinstance(alloc, mybir.MemoryLocationSet):
            continue
        name = alloc.memorylocations[0].name
        if alloc.kind == "ExternalInput":
            if name != partition_name:
                in_names.append(name)
        elif alloc.kind == "ExternalOutput":
            out_names.append(name)
            out_avals.append(jax.core.ShapedArray(
                tuple(alloc.tensor_shape), mybir.dt.np(alloc.dtype)))
    n_params = len(in_names)
    n_outs = len(out_names)
    all_names = in_names + out_names
    if partition_name is not None:
        all_names = all_names + [partition_name]
    donate = tuple(range(n_params, n_params + n_outs))

    def _body(*args):
        operands = list(args)
        if partition_name is not None:
            operands.append(bass2jax.partition_id_tensor())
        outs = bass2jax._bass_exec_p.bind(
            *operands,
            out_avals=tuple(out_avals),
            in_names=tuple(all_names),
            out_names=tuple(out_names),
            lowering_input_output_aliases=(),
            sim_require_finite=True,
            sim_require_nnan=True,
            nc=nc,
        )
        return tuple(outs)

    devices = jax.devices()[:cfg.NCORES]
    mesh = Mesh(np.asarray(devices), ("core",))
    in_specs = (PartitionSpec("core"),) * (n_params + n_outs)
    out_specs = (PartitionSpec("core"),) * n_outs
    sharded = jax.jit(
        shard_map(_body, mesh=mesh, in_specs=in_specs, out_specs=out_specs,
                  check_rep=False),
        donate_argnums=donate, keep_unused=True)

    zero_shapes = [(cfg.NCORES * a.shape[0],) + tuple(a.shape[1:])
                   for a in out_avals]
    zero_dtypes = [a.dtype for a in out_avals]
    zs = NamedSharding(mesh, PartitionSpec("core"))
    make_zeros = jax.jit(
        lambda: tuple(jnp.zeros(s, d)
                      for s, d in zip(zero_shapes, zero_dtypes)),
        out_shardings=(zs,) * n_outs)

    def run(np_inputs: dict):
        zeros = make_zeros()
        args = [np_inputs[nm] for nm in in_names] + list(zeros)
        outs = sharded(*args)
        return {nm: np.asarray(outs[i]) for i, nm in enumerate(out_names)}

    return run


def postprocess(y_g, d_g, cfg: Cfg):
    B, C, PP, WD, NH = cfg.B, cfg.C, cfg.PP, cfg.WD, cfg.NH
    NCORES = cfg.NCORES
    NCr = PP * WD
    HH = NCORES * PP
    y = y_g.reshape(NCORES, B, NH, cfg.DV, NCr)
    d = d_g.reshape(NCORES, B, NH, 1, NCr)
    out = np.empty((B, C, HH, WD), np.float32)
    ov = out.reshape(B, NH, cfg.DV, NCORES, PP, WD)

    def work(c):
        rd = 1.0 / (d[c] * (1.0 + EPS))
        yc = y[c].astype(np.float32)
        yc *= rd
        ov[:, :, :, c] = yc.reshape(B, NH, cfg.DV, PP, WD)
    list(_POOL.map(work, range(NCORES)))
    return out.reshape(B, C, HH, cfg.W, cfg.D)


def kernel(x, Wq, Wk, Wv, bv):
    cfg = Cfg()
    if "runner" not in _STATE:
        nc = build_nc(cfg)
        _STATE["runner"] = build_runner(nc, cfg)
    x = np.asarray(x, np.float32)
    inputs = prep_inputs(x, np.asarray(Wq, np.float32),
                         np.asarray(Wk, np.float32),
                         np.asarray(Wv, np.float32).reshape(cfg.C, 27),
                         np.asarray(bv, np.float32), cfg)
    outs = _STATE["runner"](inputs)
    return postprocess(outs["y"], outs["dnm"], cfg)


atexit.register(_POOL.shutdown, wait=False)


# revision 7
# speedup vs baseline: 5.1119x; 1.3830x over previous
"""nn_LinearConvAttention Trainium2 Bass kernel.

B=2, C=256, 48^3 grid, 4 heads (dqk=32, dv=64). 8 NeuronCores.

Sharding: 8-way spatial over H. Core c computes output planes [6c, 6c+6) of
both batch elements, all 256 channels. Inputs are uploaded as bf16 shards
[2, 256, 8, 48*48] (6 interior planes + 1 halo plane each side, zero-padded at
the global boundary). The only cross-core communication is a 131KB AllReduce
of the per-head kv/ksum statistics (kv contracts over the full spatial axis).

Math (per batch b):
  q = Wq x ; k = Wk x ; v = dwconv3x3x3(x) + bv
  ke = exp(k)                      (k rows are O(0.3); exp safe unshifted)
  kv[r, c] = sum_n ke[r, n] v[c, n]   ;  S[r] = sum_n ke[r, n]   (AllReduce)
  A = blockdiag_mask * kv / S[:, None]
  eq = exp(q)
  y[c, n] = sum_r A[r, c] eq[r, n]
  D[h, n] = sum_{r in h} eq[r, n]
  out = y / (D * (1 + eps))        (division folded into the host upcast;
                                    the reference's Z term == 1 exactly)
"""

import atexit
import contextlib
from concurrent.futures import ThreadPoolExecutor
from dataclasses import dataclass

import numpy as np
import ml_dtypes

import concourse.bacc as bacc
import concourse.mybir as mybir
from concourse.tile import TileContext

BF16 = mybir.dt.bfloat16
F32 = mybir.dt.float32
FP8 = mybir.dt.float8e3
I32 = mybir.dt.int32
EPS = 1e-6


@dataclass
class Cfg:
    B: int = 2
    C: int = 256
    NH: int = 4
    DQK: int = 32
    DV: int = 64
    W: int = 48
    D: int = 48
    PP: int = 6           # output planes per core
    NCORES: int = 8
    pe_ct: tuple = (True, False)   # conv unit engine per ctile: PE / DVE
    qk_chunk: int = 384
    num_chunk: int = 512

    @property
    def WD(self):
        return self.W * self.D

    @property
    def NC(self):
        return self.PP * self.WD

    @property
    def PIN(self):
        return self.PP + 2

    @property
    def DP(self):
        return self.D + 4   # padded D pitch (interior at col offset 2)

    @property
    def WDP(self):
        return (self.W + 2) * self.DP


def _tapidx(di, dj, dk):
    return (di + 1) * 9 + (dj + 1) * 3 + (dk + 1)


def build_nc(cfg: Cfg):
    WD, PIN, NC = cfg.WD, cfg.PIN, cfg.NC
    assert WD % 128 == 0

    nc = bacc.Bacc("TRN2", target_bir_lowering=False, debug=False,
                   num_devices=cfg.NCORES)

    x_in = nc.dram_tensor("x", [cfg.B, cfg.C, PIN, WD], FP8,
                          kind="ExternalInput").ap()
    wqk = nc.dram_tensor("wqk", [2, 128, 256], BF16, kind="ExternalInput").ap()
    wv = nc.dram_tensor("wv", [2, 27, 128], F32, kind="ExternalInput").ap()
    bv = nc.dram_tensor("bv", [2, 128], F32, kind="ExternalInput").ap()
    basis4 = nc.dram_tensor("basis4", [128, cfg.NH], BF16,
                            kind="ExternalInput").ap()
    bmask = nc.dram_tensor("bmask", [128, 256], BF16,
                           kind="ExternalInput").ap()
    y_out = nc.dram_tensor("y", [cfg.B, cfg.C, NC], BF16,
                           kind="ExternalOutput").ap()
    d_out = nc.dram_tensor("dnm", [cfg.B, cfg.NH, NC], F32,
                           kind="ExternalOutput").ap()
    cc_in = nc.dram_tensor("cc_in", [cfg.B, 128, 257], F32)
    cc_out = nc.dram_tensor("cc_out", [cfg.B, 128, 257], F32)

    with TileContext(nc) as tc:
        _emit(nc, tc, cfg, x_in, wqk, wv, bv, basis4, bmask, y_out, d_out,
              cc_in, cc_out)
    nc.compile()
    return nc


def _clip(s, n):
    """shift s in {-1,0,1}: returns (out_start, in_start, count)."""
    if s < 0:
        return 1, 0, n - 1
    if s > 0:
        return 0, 1, n - 1
    return 0, 0, n


def _emit(nc, tc, cfg, x_in, wqk, wv, bv, basis4, bmask, y_out, d_out,
          cc_in, cc_out):
    WD, PP, PIN, NC, W, D = cfg.WD, cfg.PP, cfg.PIN, cfg.NC, cfg.W, cfg.D
    NCH = WD // 128
    QKC = cfg.qk_chunk
    assert WD % QKC == 0
    NQK = WD // QKC
    NMC = cfg.num_chunk
    assert NC % NMC == 0
    NNM = NC // NMC
    TAPS = [(di, dj, dk) for di in (-1, 0, 1) for dj in (-1, 0, 1)
            for dk in (-1, 0, 1)]
    TAPS.remove((0, 0, 0))
    TAPS.insert(0, (0, 0, 0))  # center first: defines psum/acc init

    ctx = contextlib.ExitStack()
    with ctx:
        const_p = ctx.enter_context(tc.tile_pool(name="const", bufs=1))
        xdv_p = ctx.enter_context(tc.tile_pool(name="xdv", bufs=4))
        x8_p = ctx.enter_context(tc.tile_pool(name="x8", bufs=3))
        xod_p = ctx.enter_context(tc.tile_pool(name="xod", bufs=4))
        eq_p = ctx.enter_context(tc.tile_pool(name="eq", bufs=2))
        ke_p = ctx.enter_context(tc.tile_pool(name="ke", bufs=2))
        v_p = ctx.enter_context(tc.tile_pool(name="v", bufs=3))
        keT_p = ctx.enter_context(tc.tile_pool(name="keT", bufs=2))
        vT_p = ctx.enter_context(tc.tile_pool(name="vT", bufs=2))
        out_p = ctx.enter_context(tc.tile_pool(name="out", bufs=3))
        small_p = ctx.enter_context(tc.tile_pool(name="small", bufs=2))
        akv_p = ctx.enter_context(tc.tile_pool(name="akv", bufs=2))

        qk_ps = ctx.enter_context(tc.tile_pool(name="qkps", bufs=1, space="PSUM"))
        cv_ps = ctx.enter_context(tc.tile_pool(name="cvps", bufs=1, space="PSUM"))
        kv_ps = ctx.enter_context(tc.tile_pool(name="kvps", bufs=1, space="PSUM"))
        nm_ps = ctx.enter_context(tc.tile_pool(name="nmps", bufs=1, space="PSUM"))

        # ---- constants ----
        wqk_sb = const_p.tile([128, 2, 256], BF16, tag="wqk")
        nc.sync.dma_start(out=wqk_sb[:, :, :],
                          in_=wqk.rearrange("t c m -> c t m"))
        wv_sb = const_p.tile([128, 2, 27], F32, tag="wv")
        nc.sync.dma_start(out=wv_sb[:, :, :],
                          in_=wv.rearrange("t k c -> c t k"))
        bv_sb = const_p.tile([128, 2], F32, tag="bv")
        nc.sync.dma_start(out=bv_sb[:, :], in_=bv.rearrange("t c -> c t"))
        basis_sb = const_p.tile([128, cfg.NH], BF16, tag="basis")
        nc.sync.dma_start(out=basis_sb[:, :], in_=basis4[:, :])
        mask_sb = const_p.tile([128, 256], BF16, tag="bmask")
        nc.sync.dma_start(out=mask_sb[:, :], in_=bmask[:, :])
        ones_sb = const_p.tile([128, 1], BF16, tag="ones")
        nc.vector.memset(ones_sb[:, :], 1.0)

        # identity & per-tap diagonal weight matrices for the PE conv ctiles
        iot = const_p.tile([128, 128], I32, tag="iot")
        nc.gpsimd.iota(iot[:, :], pattern=[[1, 128]], base=0,
                       channel_multiplier=-1)
        ident = const_p.tile([128, 128], BF16, tag="ident")
        nc.vector.tensor_scalar(ident[:, :], iot[:, :], 0, None,
                                op0=mybir.AluOpType.is_equal)
        diags = {}
        for ct in range(2):
            if not cfg.pe_ct[ct]:
                continue
            dg = const_p.tile([128, 27, 128], BF16, tag=f"diag{ct}")
            for t in range(27):
                nc.vector.tensor_scalar(dg[:, t, :], ident[:, :],
                                        wv_sb[:, ct, t:t + 1], None,
                                        op0=mybir.AluOpType.mult)
            diags[ct] = dg

        # persistent padded-x ring for the PE conv ctiles (borders stay 0)
        DP, WDP = cfg.DP, cfg.WDP
        XPE_SLOTS = 5
        xpe_ring = {}
        for ct in range(2):
            if not cfg.pe_ct[ct]:
                continue
            for s in range(XPE_SLOTS):
                t = const_p.tile([128, WDP], FP8, tag=f"xpr{ct}_{s}")
                nc.vector.memset(t[:, :], 0.0)
                xpe_ring[(ct, s)] = t
        ring_ctr = {ct: 0 for ct in range(2)}

        kv_sb = {}
        eq_sl = {}

        # =============== main loop over batches ===============
        for b in range(cfg.B):
            eq_slab = eq_p.tile([128, NC], BF16, tag="eq")
            eq_sl[b] = eq_slab
            kv_tile = kv_ps.tile([128, 256], F32, tag="kv")
            kvS_tile = kv_ps.tile([128, 1], F32, tag="kvS")
            first_kv = [True]

            xs = {}
            xso = {}

            def load_plane(pl, b=b, xs=xs, xso=xso):
                for ct in range(2):
                    if (pl, ct) in xs:
                        continue
                    src_ap = x_in[b, ct * 128:(ct + 1) * 128, pl, :]
                    if cfg.pe_ct[ct]:
                        t = xpe_ring[(ct, ring_ctr[ct] % XPE_SLOTS)]
                        ring_ctr[ct] += 1
                        dst = t[:, :].rearrange("c (w d) -> c w d", d=DP)
                        nc.sync.dma_start(
                            out=dst[:, 1:W + 1, 2:D + 2],
                            in_=src_ap.rearrange("c (w d) -> c w d", d=D))
                        xs[(pl, ct)] = t
                    else:
                        st8 = x8_p.tile([128, WD], FP8, tag=f"x8{ct}")
                        nc.sync.dma_start(out=st8[:, :], in_=src_ap)
                        t = xdv_p.tile([128, WD], BF16, tag=f"x{ct}")
                        nc.scalar.copy(t[:, :], st8[:, :])
                        xs[(pl, ct)] = t
                        to = xod_p.tile([128, WD], BF16, tag=f"xo{ct}")
                        # to[:, j] = x[:, j+1]; last element garbage, never read
                        nc.scalar.copy(to[:, 0:WD - 1], st8[:, 1:WD])
                        xso[(pl, ct)] = to

            for pl in range(min(3, PIN)):
                load_plane(pl)

            for p in range(PP):
                if p + 3 < PIN:
                    load_plane(p + 3)

                # ---- q/k projections (input plane p+1) + exp ----
                ke_pl = ke_p.tile([128, WD], BF16, tag="ke")
                for ch in range(NQK):
                    qp = qk_ps.tile([128, QKC], F32, tag="qps")
                    kp = qk_ps.tile([128, QKC], F32, tag="kps")
                    rows_per_qk = QKC // D
                    for ct in range(2):
                        if cfg.pe_ct[ct]:
                            xv = xs[(p + 1, ct)][:, :].rearrange(
                                "c (w d) -> c w d", d=DP)
                            r0 = ch * rows_per_qk
                            rhs = xv[:, 1 + r0:1 + r0 + rows_per_qk, 2:D + 2]
                        else:
                            rhs = xs[(p + 1, ct)][:, ch * QKC:(ch + 1) * QKC]
                        nc.tensor.matmul(qp[:, :], wqk_sb[:, ct, 0:128], rhs,
                                         start=(ct == 0), stop=(ct == 1))
                        nc.tensor.matmul(kp[:, :], wqk_sb[:, ct, 128:256], rhs,
                                         start=(ct == 0), stop=(ct == 1))
                    nc.scalar.activation(
                        eq_slab[:, p * WD + ch * QKC:p * WD + (ch + 1) * QKC],
                        qp[:, :], mybir.ActivationFunctionType.Exp)
                    nc.scalar.activation(ke_pl[:, ch * QKC:(ch + 1) * QKC],
                                         kp[:, :],
                                         mybir.ActivationFunctionType.Exp)

                # ---- depthwise conv ----
                v_pl = {}
                for ct in range(2):
                    vt = v_p.tile([128, WD], BF16, tag=f"v{ct}")
                    v_pl[ct] = vt
                    if cfg.pe_ct[ct]:
                        _conv_pe(nc, cfg, cv_ps, vt, xs, diags[ct], bv_sb,
                                 ct, p, TAPS)
                    else:
                        _conv_dve(nc, cfg, vt, xs, xso, wv_sb, bv_sb,
                                  ct, p, TAPS)

                # ---- transposes + kv accumulation ----
                keT = keT_p.tile([128, NCH, 128], BF16, tag="keT")
                nc.sync.dma_start_transpose(keT[:, :, :], ke_pl[:, :])
                vT = vT_p.tile([128, NCH, 256], BF16, tag="vT")
                nc.sync.dma_start_transpose(vT[:, :, 0:128], v_pl[0][:, :])
                nc.scalar.dma_start_transpose(vT[:, :, 128:256], v_pl[1][:, :])
                for ch in range(NCH):
                    st = first_kv[0]
                    last = (p == PP - 1 and ch == NCH - 1)
                    nc.tensor.matmul(kv_tile[:, :], keT[:, ch, :],
                                     vT[:, ch, :], start=st, stop=last,
                                     skip_group_check=True)
                    nc.tensor.matmul(kvS_tile[:, :], keT[:, ch, :],
                                     ones_sb[:, :], start=st, stop=last,
                                     skip_group_check=True)
                    first_kv[0] = False

            # ---- AllReduce kv / S, then A = mask * kv / S ----
            kvs = akv_p.tile([128, 257], F32, tag="kvsb")
            nc.vector.tensor_copy(kvs[:, 0:256], kv_tile[:, :])
            nc.vector.tensor_copy(kvs[:, 256:257], kvS_tile[:, :])
            nc.sync.dma_start(out=cc_in.ap()[b, :, :], in_=kvs[:, :])
            nc.gpsimd.collective_compute(
                "AllReduce", mybir.AluOpType.add,
                replica_groups=[list(range(cfg.NCORES))],
                ins=[cc_in.ap()[b:b + 1, :, :].opt()],
                outs=[cc_out.ap()[b:b + 1, :, :].opt()])
            kvr = akv_p.tile([128, 257], F32, tag="kvr")
            nc.scalar.dma_start(out=kvr[:, :], in_=cc_out.ap()[b, :, :])
            rS = small_p.tile([128, 1], F32, tag="rS")
            nc.vector.reciprocal(rS[:, :], kvr[:, 256:257])
            A_sb = akv_p.tile([128, 256], BF16, tag="A")
            nc.vector.scalar_tensor_tensor(
                A_sb[:, :], kvr[:, 0:256], rS[:, 0:1], mask_sb[:, :],
                op0=mybir.AluOpType.mult, op1=mybir.AluOpType.mult)
            kv_sb[b] = A_sb

        # =============== output phase ===============
        for b in range(cfg.B):
            A_sb = kv_sb[b]
            eq_slab = eq_sl[b]
            for ch in range(NNM):
                rhs = eq_slab[:, ch * NMC:(ch + 1) * NMC]
                n0 = nm_ps.tile([128, NMC], F32, tag="n0")
                n1 = nm_ps.tile([128, NMC], F32, tag="n1")
                dr = nm_ps.tile([cfg.NH, NMC], F32, tag="dr")
                nc.tensor.matmul(n0[:, :], A_sb[:, 0:128], rhs,
                                 start=True, stop=True)
                nc.tensor.matmul(n1[:, :], A_sb[:, 128:256], rhs,
                                 start=True, stop=True)
                nc.tensor.matmul(dr[:, :], basis_sb[:, :], rhs,
                                 start=True, stop=True)
                o0 = out_p.tile([128, NMC], BF16, tag="o0")
                o1 = out_p.tile([128, NMC], BF16, tag="o1")
                nc.vector.tensor_copy(o0[:, :], n0[:, :])
                nc.scalar.copy(o1[:, :], n1[:, :])
                nc.sync.dma_start(
                    out=y_out[b, 0:128, ch * NMC:(ch + 1) * NMC], in_=o0[:, :])
                nc.sync.dma_start(
                    out=y_out[b, 128:256, ch * NMC:(ch + 1) * NMC],
                    in_=o1[:, :])
                od = out_p.tile([cfg.NH, NMC], F32, tag="od")
                nc.vector.tensor_copy(od[:, :], dr[:, :])
                nc.gpsimd.dma_start(
                    out=d_out[b, :, ch * NMC:(ch + 1) * NMC], in_=od[:, :])


def _conv_pe(nc, cfg, cv_ps, vt, xs, dg, bv_sb, ct, p, taps):
    """Conv unit on PE: per-tap diagonal-weight matmuls accumulating into
    PSUM pieces of the plane; input is the zero-padded plane tile so every
    tap is a uniform full-width window. Evicted via ACT with +bv bias."""
    W, D, DP = cfg.W, cfg.D, cfg.DP
    rows_per = max(1, 512 // D)
    n_pieces = (W + rows_per - 1) // rows_per
    for pc in range(n_pieces):
        t0, t1 = pc * rows_per, min(W, (pc + 1) * rows_per)
        nr = t1 - t0
        ps = cv_ps.tile([128, nr * D], F32, tag="cv")
        for i, (di, dj, dk) in enumerate(taps):
            xv = xs[(p + 1 + di, ct)][:, :].rearrange(
                "c (w d) -> c w d", d=DP)
            rhs = xv[:, t0 + dj + 1:t1 + dj + 1, 2 + dk:2 + dk + D]
            nc.tensor.matmul(
                ps[:, :], dg[:, _tapidx(di, dj, dk), :], rhs,
                start=(i == 0), stop=(i == len(taps) - 1),
                skip_group_check=True)
        nc.scalar.activation(
            vt[:, t0 * D:t1 * D], ps[:, :],
            mybir.ActivationFunctionType.Identity,
            bias=bv_sb[:, ct:ct + 1])


def _conv_dve(nc, cfg, vt, xs, xso, wv_sb, bv_sb, ct, p, taps):
    """Conv unit on DVE: scalar_tensor_tensor FMA into the bf16 v tile.
    D-axis (innermost) alignment for 2x mode:
      dk=0  : both APs 4B-aligned as-is
      dk=+1 : src from the odd-shifted copy xo (xo[:, j] = x[:, j+1])
      dk=-1 : dst cols [2, D) with src xo cols [0, D-2); col 1 fixed up
              with a small strided op (col 0 needs no contribution).
    """
    W, D = cfg.W, cfg.D

    def w_ap(tap):
        i = _tapidx(*tap)
        return wv_sb[:, ct, i:i + 1]

    for i, (di, dj, dk) in enumerate(taps):
        ow0, iw0, wcnt = _clip(dj, W)
        xt = xs[(p + 1 + di, ct)]
        ov = vt[:, :].rearrange("c (w d) -> c w d", d=D)
        if i == 0:
            nc.vector.tensor_scalar(
                vt[:, :], xt[:, :], w_ap((0, 0, 0)), bv_sb[:, ct:ct + 1],
                op0=mybir.AluOpType.mult, op1=mybir.AluOpType.add)
            continue
        if dk == 0:
            xv = xt[:, :].rearrange("c (w d) -> c w d", d=D)
            dst = ov[:, ow0:ow0 + wcnt, :]
            src = xv[:, iw0:iw0 + wcnt, :]
        elif dk == 1:
            xo = xso[(p + 1 + di, ct)][:, :].rearrange("c (w d) -> c w d", d=D)
            dst = ov[:, ow0:ow0 + wcnt, 0:D - 1]
            src = xo[:, iw0:iw0 + wcnt, 0:D - 1]
        else:  # dk == -1
            xo = xso[(p + 1 + di, ct)][:, :].rearrange("c (w d) -> c w d", d=D)
            dst = ov[:, ow0:ow0 + wcnt, 2:D]
            src = xo[:, iw0:iw0 + wcnt, 0:D - 2]
        nc.vector.scalar_tensor_tensor(
            dst, src, w_ap((di, dj, dk)), dst,
            op0=mybir.AluOpType.mult, op1=mybir.AluOpType.add)
        if dk == -1:
            xv = xt[:, :].rearrange("c (w d) -> c w d", d=D)
            d1 = ov[:, ow0:ow0 + wcnt, 1:2]
            s0 = xv[:, iw0:iw0 + wcnt, 0:1]
            nc.vector.scalar_tensor_tensor(
                d1, s0, w_ap((di, dj, dk)), d1,
                op0=mybir.AluOpType.mult, op1=mybir.AluOpType.add)


# ======================================================================
# host side
# ======================================================================

_STATE = {}
_POOL = ThreadPoolExecutor(16)


def _mt_copy(dst, src):
    n = dst.shape[0]
    cs = max(1, (n + 15) // 16)

    def work(i):
        dst[i * cs:(i + 1) * cs] = src[i * cs:(i + 1) * cs]
    list(_POOL.map(work, range((n + cs - 1) // cs)))


def prep_inputs(x, Wq, Wk, Wv27, bvec, cfg: Cfg):
    B, C, PP, PIN, WD = cfg.B, cfg.C, cfg.PP, cfg.PIN, cfg.WD
    NCORES = cfg.NCORES
    HH = NCORES * PP

    xr = np.ascontiguousarray(x.reshape(B, C, HH, WD))
    xpad = np.zeros((B, C, HH + 2, WD), ml_dtypes.float8_e3m4)
    _mt_copy(xpad.reshape(B * C, HH + 2, WD)[:, 1:HH + 1],
             xr.reshape(B * C, HH, WD))
    s = xpad.strides
    Gv = np.lib.stride_tricks.as_strided(
        xpad, (NCORES, B, C, PIN, WD), (PP * s[2], s[0], s[1], s[2], s[3]))
    G = np.empty((NCORES, B, C, PIN, WD), ml_dtypes.float8_e3m4)
    _mt_copy(G, Gv)
    x_global = G.reshape(NCORES * B, C, PIN, WD)

    wqk1 = np.empty((2, 128, 256), ml_dtypes.bfloat16)
    for ct in range(2):
        wqk1[ct, :, 0:128] = Wq[:, ct * 128:(ct + 1) * 128].T
        wqk1[ct, :, 128:256] = Wk[:, ct * 128:(ct + 1) * 128].T
    wv1 = np.empty((2, 27, 128), np.float32)
    for ct in range(2):
        wv1[ct] = Wv27[ct * 128:(ct + 1) * 128].T
    bv1 = bvec.reshape(2, 128).astype(np.float32)
    basis = np.zeros((128, cfg.NH), ml_dtypes.bfloat16)
    for r in range(128):
        basis[r, r // cfg.DQK] = 1
    bm = np.zeros((128, 256), ml_dtypes.bfloat16)
    for r in range(128):
        h = r // cfg.DQK
        bm[r, h * cfg.DV:(h + 1) * cfg.DV] = 1
    rep = lambda a: np.broadcast_to(
        a[None], (NCORES,) + a.shape).reshape((NCORES * a.shape[0],) +
                                              a.shape[1:])
    return {
        "x": x_global,
        "wqk": np.ascontiguousarray(rep(wqk1)),
        "wv": np.ascontiguousarray(rep(wv1)),
        "bv": np.ascontiguousarray(rep(bv1)),
        "basis4": np.ascontiguousarray(rep(basis)),
        "bmask": np.ascontiguousarray(rep(bm)),
    }


def build_runner(nc, cfg: Cfg):
    """One shard_map'd jit over 8 cores; donated output buffers are created
    on-device (the stock run_bass_via_pjrt uploads host zeros every call)."""
    import jax
    import jax.numpy as jnp
    from jax.experimental.shard_map import shard_map
    from jax.sharding import Mesh, PartitionSpec, NamedSharding
    from concourse import bass2jax

    bass2jax.install_neuronx_cc_hook()

    partition_name = (nc.partition_id_tensor.name
                      if nc.partition_id_tensor else None)
    in_names, out_names, out_avals = [], [], []
    for alloc in nc.m.functions[0].allocations:
        if not isinstance(alloc, mybir.MemoryLocationSet):
            continue
        name = alloc.memorylocations[0].name
        if alloc.kind == "ExternalInput":
            if name != partition_name:
                in_names.append(name)
        elif alloc.kind == "ExternalOutput":
            out_names.append(name)
            out_avals.append(jax.core.ShapedArray(
                tuple(alloc.tensor_shape), mybir.dt.np(alloc.dtype)))
    n_params = len(in_names)
    n_outs = len(out_names)
    all_names = in_names + out_names
    if partition_name is not None:
        all_names = all_names + [partition_name]
    donate = tuple(range(n_params, n_params + n_outs))

    def _body(*args):
        operands = list(args)
        if partition_name is not None:
            operands.append(bass2jax.partition_id_tensor())
        outs = bass2jax._bass_exec_p.bind(
            *operands,
            out_avals=tuple(out_avals),
            in_names=tuple(all_names),
            out_names=tuple(out_names),
            lowering_input_output_aliases=(),
            sim_require_finite=True,
            sim_require_nnan=True,
            nc=nc,
        )
        return tuple(outs)

    devices = jax.devices()[:cfg.NCORES]
    mesh = Mesh(np.asarray(devices), ("core",))
    in_specs = (PartitionSpec("core"),) * (n_params + n_outs)
    out_specs = (PartitionSpec("core"),) * n_outs
    sharded = jax.jit(
        shard_map(_body, mesh=mesh, in_specs=in_specs, out_specs=out_specs,
                  check_rep=False),
        donate_argnums=donate, keep_unused=True)

    zero_shapes = [(cfg.NCORES * a.shape[0],) + tuple(a.shape[1:])
                   for a in out_avals]
    zero_dtypes = [a.dtype for a in out_avals]
    zs = NamedSharding(mesh, PartitionSpec("core"))
    make_zeros = jax.jit(
        lambda: tuple(jnp.zeros(s, d)
                      for s, d in zip(zero_shapes, zero_dtypes)),
        out_shardings=(zs,) * n_outs)

    state = {"donate": None}

    def run(np_inputs: dict):
        donate_bufs = state["donate"]
        if donate_bufs is None:
            donate_bufs = make_zeros()
        args = [np_inputs[nm] for nm in in_names] + list(donate_bufs)
        outs = sharded(*args)
        # next call reuses these buffers as (donated) outputs; the kernel
        # overwrites every element so they need not be zero.
        state["donate"] = outs
        for o in outs:
            o.copy_to_host_async()
        return {nm: np.asarray(outs[i]) for i, nm in enumerate(out_names)}

    return run


def postprocess(y_g, d_g, cfg: Cfg):
    B, C, PP, WD, NH = cfg.B, cfg.C, cfg.PP, cfg.WD, cfg.NH
    NCORES = cfg.NCORES
    NCr = PP * WD
    HH = NCORES * PP
    y = y_g.reshape(NCORES, B, NH, cfg.DV, NCr)
    d = d_g.reshape(NCORES, B, NH, 1, NCr)
    out = np.empty((B, C, HH, WD), np.float32)
    ov = out.reshape(B, NH, cfg.DV, NCORES, PP, WD)

    def work(c):
        rd = 1.0 / (d[c] * (1.0 + EPS))
        yc = y[c].astype(np.float32)
        yc *= rd
        ov[:, :, :, c] = yc.reshape(B, NH, cfg.DV, PP, WD)
    list(_POOL.map(work, range(NCORES)))
    return out.reshape(B, C, HH, cfg.W, cfg.D)


def kernel(x, Wq, Wk, Wv, bv):
    cfg = Cfg()
    if "runner" not in _STATE:
        nc = build_nc(cfg)
        _STATE["runner"] = build_runner(nc, cfg)
    x = np.asarray(x, np.float32)
    inputs = prep_inputs(x, np.asarray(Wq, np.float32),
                         np.asarray(Wk, np.float32),
                         np.asarray(Wv, np.float32).reshape(cfg.C, 27),
                         np.asarray(bv, np.float32), cfg)
    outs = _STATE["runner"](inputs)
    return postprocess(outs["y"], outs["dnm"], cfg)


atexit.register(_POOL.shutdown, wait=False)


# revision 9
# speedup vs baseline: 6.0538x; 1.1842x over previous
"""nn_LinearConvAttention Trainium2 Bass kernel.

B=2, C=256, 48^3 grid, 4 heads (dqk=32, dv=64). 8 NeuronCores.

Sharding: 8-way spatial over H. Core c computes output planes [6c, 6c+6) of
both batch elements, all 256 channels. Inputs are uploaded as bf16 shards
[2, 256, 8, 48*48] (6 interior planes + 1 halo plane each side, zero-padded at
the global boundary). The only cross-core communication is a 131KB AllReduce
of the per-head kv/ksum statistics (kv contracts over the full spatial axis).

Math (per batch b):
  q = Wq x ; k = Wk x ; v = dwconv3x3x3(x) + bv
  ke = exp(k)                      (k rows are O(0.3); exp safe unshifted)
  kv[r, c] = sum_n ke[r, n] v[c, n]   ;  S[r] = sum_n ke[r, n]   (AllReduce)
  A = blockdiag_mask * kv / S[:, None]
  eq = exp(q)
  y[c, n] = sum_r A[r, c] eq[r, n]
  D[h, n] = sum_{r in h} eq[r, n]
  out = y / (D * (1 + eps))        (division folded into the host upcast;
                                    the reference's Z term == 1 exactly)
"""

import atexit
import contextlib
from concurrent.futures import ThreadPoolExecutor
from dataclasses import dataclass

import numpy as np
import ml_dtypes

import concourse.bacc as bacc
import concourse.mybir as mybir
from concourse.tile import TileContext

BF16 = mybir.dt.bfloat16
F32 = mybir.dt.float32
FP8 = mybir.dt.float8e3
I32 = mybir.dt.int32
EPS = 1e-6


@dataclass
class Cfg:
    B: int = 2
    C: int = 256
    NH: int = 4
    DQK: int = 32
    DV: int = 64
    W: int = 48
    D: int = 48
    PP: int = 6           # output planes per core
    NCORES: int = 8
    pe_ct: tuple = (True, False)   # conv unit engine per ctile: PE / DVE
    qk_chunk: int = 384
    num_chunk: int = 512

    @property
    def WD(self):
        return self.W * self.D

    @property
    def NC(self):
        return self.PP * self.WD

    @property
    def PIN(self):
        return self.PP + 2

    @property
    def DP(self):
        return self.D + 4   # padded D pitch (interior at col offset 2)

    @property
    def WDP(self):
        return (self.W + 2) * self.DP


def _tapidx(di, dj, dk):
    return (di + 1) * 9 + (dj + 1) * 3 + (dk + 1)


def build_nc(cfg: Cfg):
    WD, PIN, NC = cfg.WD, cfg.PIN, cfg.NC
    assert WD % 128 == 0

    nc = bacc.Bacc("TRN2", target_bir_lowering=False, debug=False,
                   num_devices=cfg.NCORES)

    x_in = nc.dram_tensor("x", [cfg.B, cfg.C, PIN, WD], FP8,
                          kind="ExternalInput").ap()
    wqk = nc.dram_tensor("wqk", [2, 128, 256], BF16, kind="ExternalInput").ap()
    wv = nc.dram_tensor("wv", [2, 27, 128], F32, kind="ExternalInput").ap()
    bv = nc.dram_tensor("bv", [2, 128], F32, kind="ExternalInput").ap()
    basis4 = nc.dram_tensor("basis4", [128, cfg.NH], BF16,
                            kind="ExternalInput").ap()
    bmask = nc.dram_tensor("bmask", [128, 256], BF16,
                           kind="ExternalInput").ap()
    y_out = nc.dram_tensor("y", [cfg.B, cfg.C, NC], mybir.dt.int8,
                           kind="ExternalOutput").ap()
    sc_out = nc.dram_tensor("ysc", [cfg.B, 128, 2, NC // cfg.num_chunk], F32,
                            kind="ExternalOutput").ap()
    d_out = nc.dram_tensor("dnm", [cfg.B, cfg.NH, NC], F32,
                           kind="ExternalOutput").ap()
    cc_in = nc.dram_tensor("cc_in", [cfg.B, 128, 257], F32)
    drm_dram = nc.dram_tensor("drm_b", [4, 1], F32)
    cc_out = nc.dram_tensor("cc_out", [cfg.B, 128, 257], F32)

    with TileContext(nc) as tc:
        _emit(nc, tc, cfg, x_in, wqk, wv, bv, basis4, bmask, y_out, d_out,
              cc_in, cc_out, sc_out, drm_dram)
    nc.compile()
    return nc


def _clip(s, n):
    """shift s in {-1,0,1}: returns (out_start, in_start, count)."""
    if s < 0:
        return 1, 0, n - 1
    if s > 0:
        return 0, 1, n - 1
    return 0, 0, n


def _emit(nc, tc, cfg, x_in, wqk, wv, bv, basis4, bmask, y_out, d_out,
          cc_in, cc_out, sc_out, drm_dram):
    WD, PP, PIN, NC, W, D = cfg.WD, cfg.PP, cfg.PIN, cfg.NC, cfg.W, cfg.D
    NCH = WD // 128
    QKC = cfg.qk_chunk
    assert WD % QKC == 0
    NQK = WD // QKC
    NMC = cfg.num_chunk
    assert NC % NMC == 0
    NNM = NC // NMC
    TAPS = [(di, dj, dk) for di in (-1, 0, 1) for dj in (-1, 0, 1)
            for dk in (-1, 0, 1)]
    TAPS.remove((0, 0, 0))
    TAPS.insert(0, (0, 0, 0))  # center first: defines psum/acc init

    ctx = contextlib.ExitStack()
    with ctx:
        const_p = ctx.enter_context(tc.tile_pool(name="const", bufs=1))
        xdv_p = ctx.enter_context(tc.tile_pool(name="xdv", bufs=4))
        x8_p = ctx.enter_context(tc.tile_pool(name="x8", bufs=3))
        xod_p = ctx.enter_context(tc.tile_pool(name="xod", bufs=4))
        eq_p = ctx.enter_context(tc.tile_pool(name="eq", bufs=2))
        ke_p = ctx.enter_context(tc.tile_pool(name="ke", bufs=2))
        v_p = ctx.enter_context(tc.tile_pool(name="v", bufs=3))
        keT_p = ctx.enter_context(tc.tile_pool(name="keT", bufs=2))
        vT_p = ctx.enter_context(tc.tile_pool(name="vT", bufs=2))
        out_p = ctx.enter_context(tc.tile_pool(name="out", bufs=3))
        small_p = ctx.enter_context(tc.tile_pool(name="small", bufs=2))
        akv_p = ctx.enter_context(tc.tile_pool(name="akv", bufs=2))

        qk_ps = ctx.enter_context(tc.tile_pool(name="qkps", bufs=1, space="PSUM"))
        cv_ps = ctx.enter_context(tc.tile_pool(name="cvps", bufs=1, space="PSUM"))
        kv_ps = ctx.enter_context(tc.tile_pool(name="kvps", bufs=1, space="PSUM"))
        nm_ps = ctx.enter_context(tc.tile_pool(name="nmps", bufs=1, space="PSUM"))

        # ---- constants ----
        wqk_sb = const_p.tile([128, 2, 256], BF16, tag="wqk")
        nc.sync.dma_start(out=wqk_sb[:, :, :],
                          in_=wqk.rearrange("t c m -> c t m"))
        wv_sb = const_p.tile([128, 2, 27], F32, tag="wv")
        nc.sync.dma_start(out=wv_sb[:, :, :],
                          in_=wv.rearrange("t k c -> c t k"))
        bv_sb = const_p.tile([128, 2], F32, tag="bv")
        nc.sync.dma_start(out=bv_sb[:, :], in_=bv.rearrange("t c -> c t"))
        basis_sb = const_p.tile([128, cfg.NH], BF16, tag="basis")
        nc.sync.dma_start(out=basis_sb[:, :], in_=basis4[:, :])
        mask_sb = const_p.tile([128, 256], BF16, tag="bmask")
        nc.sync.dma_start(out=mask_sb[:, :], in_=bmask[:, :])
        ones_sb = const_p.tile([128, 1], BF16, tag="ones")
        nc.vector.memset(ones_sb[:, :], 1.0)

        # identity & per-tap diagonal weight matrices for the PE conv ctiles
        iot = const_p.tile([128, 128], I32, tag="iot")
        nc.gpsimd.iota(iot[:, :], pattern=[[1, 128]], base=0,
                       channel_multiplier=-1)
        ident = const_p.tile([128, 128], BF16, tag="ident")
        nc.vector.tensor_scalar(ident[:, :], iot[:, :], 0, None,
                                op0=mybir.AluOpType.is_equal)
        diags = {}
        for ct in range(2):
            if not cfg.pe_ct[ct]:
                continue
            dg = const_p.tile([128, 27, 128], BF16, tag=f"diag{ct}")
            for t in range(27):
                nc.vector.tensor_scalar(dg[:, t, :], ident[:, :],
                                        wv_sb[:, ct, t:t + 1], None,
                                        op0=mybir.AluOpType.mult)
            diags[ct] = dg

        # persistent padded-x ring for the PE conv ctiles (borders stay 0)
        DP, WDP = cfg.DP, cfg.WDP
        XPE_SLOTS = 5
        xpe_ring = {}
        for ct in range(2):
            if not cfg.pe_ct[ct]:
                continue
            for s in range(XPE_SLOTS):
                t = const_p.tile([128, WDP], FP8, tag=f"xpr{ct}_{s}")
                nc.vector.memset(t[:, :], 0.0)
                xpe_ring[(ct, s)] = t
        ring_ctr = {ct: 0 for ct in range(2)}

        kv_sb = {}
        eq_sl = {}

        # =============== main loop over batches ===============
        for b in range(cfg.B):
            eq_slab = eq_p.tile([128, NC], BF16, tag="eq")
            eq_sl[b] = eq_slab
            kv_tile = kv_ps.tile([128, 256], F32, tag="kv")
            kvS_tile = kv_ps.tile([128, 1], F32, tag="kvS")
            first_kv = [True]

            xs = {}
            xso = {}

            def load_plane(pl, b=b, xs=xs, xso=xso):
                for ct in range(2):
                    if (pl, ct) in xs:
                        continue
                    src_ap = x_in[b, ct * 128:(ct + 1) * 128, pl, :]
                    if cfg.pe_ct[ct]:
                        t = xpe_ring[(ct, ring_ctr[ct] % XPE_SLOTS)]
                        ring_ctr[ct] += 1
                        dst = t[:, :].rearrange("c (w d) -> c w d", d=DP)
                        nc.sync.dma_start(
                            out=dst[:, 1:W + 1, 2:D + 2],
                            in_=src_ap.rearrange("c (w d) -> c w d", d=D))
                        xs[(pl, ct)] = t
                    else:
                        st8 = x8_p.tile([128, WD], FP8, tag=f"x8{ct}")
                        nc.sync.dma_start(out=st8[:, :], in_=src_ap)
                        t = xdv_p.tile([128, WD], BF16, tag=f"x{ct}")
                        nc.scalar.copy(t[:, :], st8[:, :])
                        xs[(pl, ct)] = t
                        to = xod_p.tile([128, WD], BF16, tag=f"xo{ct}")
                        # to[:, j] = x[:, j+1]; last element garbage, never read
                        nc.scalar.copy(to[:, 0:WD - 1], st8[:, 1:WD])
                        xso[(pl, ct)] = to

            for pl in range(min(3, PIN)):
                load_plane(pl)

            for p in range(PP):
                if p + 3 < PIN:
                    load_plane(p + 3)

                # ---- q/k projections (input plane p+1) + exp ----
                ke_pl = ke_p.tile([128, WD], BF16, tag="ke")
                for ch in range(NQK):
                    qp = qk_ps.tile([128, QKC], F32, tag="qps")
                    kp = qk_ps.tile([128, QKC], F32, tag="kps")
                    rows_per_qk = QKC // D
                    for ct in range(2):
                        if cfg.pe_ct[ct]:
                            xv = xs[(p + 1, ct)][:, :].rearrange(
                                "c (w d) -> c w d", d=DP)
                            r0 = ch * rows_per_qk
                            rhs = xv[:, 1 + r0:1 + r0 + rows_per_qk, 2:D + 2]
                        else:
                            rhs = xs[(p + 1, ct)][:, ch * QKC:(ch + 1) * QKC]
                        nc.tensor.matmul(qp[:, :], wqk_sb[:, ct, 0:128], rhs,
                                         start=(ct == 0), stop=(ct == 1))
                        nc.tensor.matmul(kp[:, :], wqk_sb[:, ct, 128:256], rhs,
                                         start=(ct == 0), stop=(ct == 1))
                    nc.scalar.activation(
                        eq_slab[:, p * WD + ch * QKC:p * WD + (ch + 1) * QKC],
                        qp[:, :], mybir.ActivationFunctionType.Exp)
                    nc.scalar.activation(ke_pl[:, ch * QKC:(ch + 1) * QKC],
                                         kp[:, :],
                                         mybir.ActivationFunctionType.Exp)

                # ---- depthwise conv ----
                v_pl = {}
                for ct in range(2):
                    vt = v_p.tile([128, WD], BF16, tag=f"v{ct}")
                    v_pl[ct] = vt
                    if cfg.pe_ct[ct]:
                        _conv_pe(nc, cfg, cv_ps, vt, xs, diags[ct], bv_sb,
                                 ct, p, TAPS)
                    else:
                        _conv_dve(nc, cfg, vt, xs, xso, wv_sb, bv_sb,
                                  ct, p, TAPS)

                # ---- transposes + kv accumulation ----
                keT = keT_p.tile([128, NCH, 128], BF16, tag="keT")
                nc.sync.dma_start_transpose(keT[:, :, :], ke_pl[:, :])
                vT = vT_p.tile([128, NCH, 256], BF16, tag="vT")
                nc.sync.dma_start_transpose(vT[:, :, 0:128], v_pl[0][:, :])
                nc.scalar.dma_start_transpose(vT[:, :, 128:256], v_pl[1][:, :])
                for ch in range(NCH):
                    st = first_kv[0]
                    last = (p == PP - 1 and ch == NCH - 1)
                    nc.tensor.matmul(kv_tile[:, :], keT[:, ch, :],
                                     vT[:, ch, :], start=st, stop=last,
                                     skip_group_check=True)
                    nc.tensor.matmul(kvS_tile[:, :], keT[:, ch, :],
                                     ones_sb[:, :], start=st, stop=last,
                                     skip_group_check=True)
                    first_kv[0] = False

            # ---- AllReduce kv / S, then A = mask * kv / S ----
            kvs = akv_p.tile([128, 257], F32, tag="kvsb")
            nc.vector.tensor_copy(kvs[:, 0:256], kv_tile[:, :])
            nc.vector.tensor_copy(kvs[:, 256:257], kvS_tile[:, :])
            nc.sync.dma_start(out=cc_in.ap()[b, :, :], in_=kvs[:, :])
            nc.gpsimd.collective_compute(
                "AllReduce", mybir.AluOpType.add,
                replica_groups=[list(range(cfg.NCORES))],
                ins=[cc_in.ap()[b:b + 1, :, :].opt()],
                outs=[cc_out.ap()[b:b + 1, :, :].opt()])
            kvr = akv_p.tile([128, 257], F32, tag="kvr")
            nc.scalar.dma_start(out=kvr[:, :], in_=cc_out.ap()[b, :, :])
            rS = small_p.tile([128, 1], F32, tag="rS")
            nc.vector.reciprocal(rS[:, :], kvr[:, 256:257])
            A_sb = akv_p.tile([128, 256], BF16, tag="A")
            nc.vector.scalar_tensor_tensor(
                A_sb[:, :], kvr[:, 0:256], rS[:, 0:1], mask_sb[:, :],
                op0=mybir.AluOpType.mult, op1=mybir.AluOpType.mult)
            kv_sb[b] = A_sb

        # =============== output phase ===============
        for b in range(cfg.B):
            A_sb = kv_sb[b]
            eq_slab = eq_sl[b]
            # |y[c,n]| <= max_r |A[r,c]| * D[h(c),n]: per-channel |A| max via
            # xbar transpose (partition-aligned with the output tiles).
            AT = akv_p.tile([128, 2, 128], BF16, tag="AT")
            nc.sync.dma_start_transpose(AT[:, :, :], A_sb[:, :])
            maxA = small_p.tile([128, 2], F32, tag="maxA")
            for ct in range(2):
                nc.vector.tensor_reduce(
                    maxA[:, ct:ct + 1], AT[:, ct, :],
                    axis=mybir.AxisListType.X, op=mybir.AluOpType.max,
                    apply_absolute_value=True)
            # pre-divide by the int8 target amplitude
            maxAp = small_p.tile([128, 2], F32, tag="maxAp")
            nc.vector.tensor_scalar(maxAp[:, :], maxA[:, :], 1.0 / 126.0,
                                    None, op0=mybir.AluOpType.mult)
            sc_stage = out_p.tile([128, 2, NNM], F32, tag="scst")
            for ch in range(NNM):
                rhs = eq_slab[:, ch * NMC:(ch + 1) * NMC]
                n0 = nm_ps.tile([128, NMC], F32, tag="n0")
                n1 = nm_ps.tile([128, NMC], F32, tag="n1")
                dr = nm_ps.tile([cfg.NH, NMC], F32, tag="dr")
                nc.tensor.matmul(n0[:, :], A_sb[:, 0:128], rhs,
                                 start=True, stop=True)
                nc.tensor.matmul(n1[:, :], A_sb[:, 128:256], rhs,
                                 start=True, stop=True)
                nc.tensor.matmul(dr[:, :], basis_sb[:, :], rhs,
                                 start=True, stop=True)
                # per-head chunk max of D, broadcast to all 128 partitions
                # via a DRAM bounce with a stride-0 read
                drm = small_p.tile([cfg.NH, 1], F32, tag="drm")
                nc.vector.tensor_reduce(drm[:, :], dr[:, :],
                                        axis=mybir.AxisListType.X,
                                        op=mybir.AluOpType.max)
                nc.gpsimd.dma_start(out=drm_dram.ap()[:, :], in_=drm[:, :])
                drb = small_p.tile([128, 1], F32, tag="drb")
                nc.gpsimd.dma_start(
                    out=drb[:, :],
                    in_=drm_dram.ap().broadcast_to([cfg.NH, 128 // cfg.NH, 1]))
                for ct in range(2):
                    nc.vector.tensor_scalar(
                        sc_stage[:, ct, ch:ch + 1], drb[:, :],
                        maxAp[:, ct:ct + 1], None, op0=mybir.AluOpType.mult)
                    s = small_p.tile([128, 1], F32, tag=f"s{ct}")
                    nc.vector.reciprocal(s[:, :], sc_stage[:, ct, ch:ch + 1])
                    oi = out_p.tile([128, NMC], mybir.dt.int8, tag=f"oi{ct}")
                    nt = n0 if ct == 0 else n1
                    if ct == 0:
                        nc.vector.tensor_scalar(
                            oi[:, :], nt[:, :], s[:, 0:1], None,
                            op0=mybir.AluOpType.mult)
                    else:
                        nc.scalar.activation(
                            oi[:, :], nt[:, :],
                            mybir.ActivationFunctionType.Copy,
                            scale=s[:, 0:1])
                    nc.sync.dma_start(
                        out=y_out[b, ct * 128:(ct + 1) * 128,
                                  ch * NMC:(ch + 1) * NMC],
                        in_=oi[:, :])
                od = out_p.tile([cfg.NH, NMC], F32, tag="od")
                nc.vector.tensor_copy(od[:, :], dr[:, :])
                nc.gpsimd.dma_start(
                    out=d_out[b, :, ch * NMC:(ch + 1) * NMC], in_=od[:, :])
            nc.sync.dma_start(out=sc_out[b, :, :, :], in_=sc_stage[:, :, :])


def _conv_pe(nc, cfg, cv_ps, vt, xs, dg, bv_sb, ct, p, taps):
    """Conv unit on PE: per-tap diagonal-weight matmuls accumulating into
    PSUM pieces of the plane; input is the zero-padded plane tile so every
    tap is a uniform full-width window. Evicted via ACT with +bv bias."""
    W, D, DP = cfg.W, cfg.D, cfg.DP
    rows_per = max(1, 512 // D)
    n_pieces = (W + rows_per - 1) // rows_per
    for pc in range(n_pieces):
        t0, t1 = pc * rows_per, min(W, (pc + 1) * rows_per)
        nr = t1 - t0
        ps = cv_ps.tile([128, nr * D], F32, tag="cv")
        for i, (di, dj, dk) in enumerate(taps):
            xv = xs[(p + 1 + di, ct)][:, :].rearrange(
                "c (w d) -> c w d", d=DP)
            rhs = xv[:, t0 + dj + 1:t1 + dj + 1, 2 + dk:2 + dk + D]
            nc.tensor.matmul(
                ps[:, :], dg[:, _tapidx(di, dj, dk), :], rhs,
                start=(i == 0), stop=(i == len(taps) - 1),
                skip_group_check=True)
        nc.scalar.activation(
            vt[:, t0 * D:t1 * D], ps[:, :],
            mybir.ActivationFunctionType.Identity,
            bias=bv_sb[:, ct:ct + 1])


def _conv_dve(nc, cfg, vt, xs, xso, wv_sb, bv_sb, ct, p, taps):
    """Conv unit on DVE: scalar_tensor_tensor FMA into the bf16 v tile.
    D-axis (innermost) alignment for 2x mode:
      dk=0  : both APs 4B-aligned as-is
      dk=+1 : src from the odd-shifted copy xo (xo[:, j] = x[:, j+1])
      dk=-1 : dst cols [2, D) with src xo cols [0, D-2); col 1 fixed up
              with a small strided op (col 0 needs no contribution).
    """
    W, D = cfg.W, cfg.D

    def w_ap(tap):
        i = _tapidx(*tap)
        return wv_sb[:, ct, i:i + 1]

    for i, (di, dj, dk) in enumerate(taps):
        ow0, iw0, wcnt = _clip(dj, W)
        xt = xs[(p + 1 + di, ct)]
        ov = vt[:, :].rearrange("c (w d) -> c w d", d=D)
        if i == 0:
            nc.vector.tensor_scalar(
                vt[:, :], xt[:, :], w_ap((0, 0, 0)), bv_sb[:, ct:ct + 1],
                op0=mybir.AluOpType.mult, op1=mybir.AluOpType.add)
            continue
        if dk == 0:
            xv = xt[:, :].rearrange("c (w d) -> c w d", d=D)
            dst = ov[:, ow0:ow0 + wcnt, :]
            src = xv[:, iw0:iw0 + wcnt, :]
        elif dk == 1:
            xo = xso[(p + 1 + di, ct)][:, :].rearrange("c (w d) -> c w d", d=D)
            dst = ov[:, ow0:ow0 + wcnt, 0:D - 1]
            src = xo[:, iw0:iw0 + wcnt, 0:D - 1]
        else:  # dk == -1
            xo = xso[(p + 1 + di, ct)][:, :].rearrange("c (w d) -> c w d", d=D)
            dst = ov[:, ow0:ow0 + wcnt, 2:D]
            src = xo[:, iw0:iw0 + wcnt, 0:D - 2]
        nc.vector.scalar_tensor_tensor(
            dst, src, w_ap((di, dj, dk)), dst,
            op0=mybir.AluOpType.mult, op1=mybir.AluOpType.add)
        if dk == -1:
            xv = xt[:, :].rearrange("c (w d) -> c w d", d=D)
            d1 = ov[:, ow0:ow0 + wcnt, 1:2]
            s0 = xv[:, iw0:iw0 + wcnt, 0:1]
            nc.vector.scalar_tensor_tensor(
                d1, s0, w_ap((di, dj, dk)), d1,
                op0=mybir.AluOpType.mult, op1=mybir.AluOpType.add)


# ======================================================================
# host side
# ======================================================================

_STATE = {}
_POOL = ThreadPoolExecutor(16)


def _mt_copy(dst, src):
    n = dst.shape[0]
    cs = max(1, (n + 15) // 16)

    def work(i):
        dst[i * cs:(i + 1) * cs] = src[i * cs:(i + 1) * cs]
    list(_POOL.map(work, range((n + cs - 1) // cs)))


def prep_inputs(x, Wq, Wk, Wv27, bvec, cfg: Cfg):
    B, C, PP, PIN, WD = cfg.B, cfg.C, cfg.PP, cfg.PIN, cfg.WD
    NCORES = cfg.NCORES
    HH = NCORES * PP

    xr = np.ascontiguousarray(x.reshape(B, C, HH, WD))
    xpad = np.zeros((B, C, HH + 2, WD), ml_dtypes.float8_e3m4)
    _mt_copy(xpad.reshape(B * C, HH + 2, WD)[:, 1:HH + 1],
             xr.reshape(B * C, HH, WD))
    s = xpad.strides
    Gv = np.lib.stride_tricks.as_strided(
        xpad, (NCORES, B, C, PIN, WD), (PP * s[2], s[0], s[1], s[2], s[3]))
    G = np.empty((NCORES, B, C, PIN, WD), ml_dtypes.float8_e3m4)
    _mt_copy(G, Gv)
    x_global = G.reshape(NCORES * B, C, PIN, WD)

    wqk1 = np.empty((2, 128, 256), ml_dtypes.bfloat16)
    for ct in range(2):
        wqk1[ct, :, 0:128] = Wq[:, ct * 128:(ct + 1) * 128].T
        wqk1[ct, :, 128:256] = Wk[:, ct * 128:(ct + 1) * 128].T
    wv1 = np.empty((2, 27, 128), np.float32)
    for ct in range(2):
        wv1[ct] = Wv27[ct * 128:(ct + 1) * 128].T
    bv1 = bvec.reshape(2, 128).astype(np.float32)
    basis = np.zeros((128, cfg.NH), ml_dtypes.bfloat16)
    for r in range(128):
        basis[r, r // cfg.DQK] = 1
    bm = np.zeros((128, 256), ml_dtypes.bfloat16)
    for r in range(128):
        h = r // cfg.DQK
        bm[r, h * cfg.DV:(h + 1) * cfg.DV] = 1
    rep = lambda a: np.broadcast_to(
        a[None], (NCORES,) + a.shape).reshape((NCORES * a.shape[0],) +
                                              a.shape[1:])
    return {
        "x": x_global,
        "wqk": np.ascontiguousarray(rep(wqk1)),
        "wv": np.ascontiguousarray(rep(wv1)),
        "bv": np.ascontiguousarray(rep(bv1)),
        "basis4": np.ascontiguousarray(rep(basis)),
        "bmask": np.ascontiguousarray(rep(bm)),
    }


def build_runner(nc, cfg: Cfg):
    """One shard_map'd jit over 8 cores; donated output buffers are created
    on-device (the stock run_bass_via_pjrt uploads host zeros every call)."""
    import jax
    import jax.numpy as jnp
    from jax.experimental.shard_map import shard_map
    from jax.sharding import Mesh, PartitionSpec, NamedSharding
    from concourse import bass2jax

    bass2jax.install_neuronx_cc_hook()

    partition_name = (nc.partition_id_tensor.name
                      if nc.partition_id_tensor else None)
    in_names, out_names, out_avals = [], [], []
    for alloc in nc.m.functions[0].allocations:
        if not isinstance(alloc, mybir.MemoryLocationSet):
            continue
        name = alloc.memorylocations[0].name
        if alloc.kind == "ExternalInput":
            if name != partition_name:
                in_names.append(name)
        elif alloc.kind == "ExternalOutput":
            out_names.append(name)
            out_avals.append(jax.core.ShapedArray(
                tuple(alloc.tensor_shape), mybir.dt.np(alloc.dtype)))
    n_params = len(in_names)
    n_outs = len(out_names)
    all_names = in_names + out_names
    if partition_name is not None:
        all_names = all_names + [partition_name]
    donate = tuple(range(n_params, n_params + n_outs))

    def _body(*args):
        operands = list(args)
        if partition_name is not None:
            operands.append(bass2jax.partition_id_tensor())
        outs = bass2jax._bass_exec_p.bind(
            *operands,
            out_avals=tuple(out_avals),
            in_names=tuple(all_names),
            out_names=tuple(out_names),
            lowering_input_output_aliases=(),
            sim_require_finite=True,
            sim_require_nnan=True,
            nc=nc,
        )
        return tuple(outs)

    devices = jax.devices()[:cfg.NCORES]
    mesh = Mesh(np.asarray(devices), ("core",))
    in_specs = (PartitionSpec("core"),) * (n_params + n_outs)
    out_specs = (PartitionSpec("core"),) * n_outs
    sharded = jax.jit(
        shard_map(_body, mesh=mesh, in_specs=in_specs, out_specs=out_specs,
                  check_rep=False),
        donate_argnums=donate, keep_unused=True)

    zero_shapes = [(cfg.NCORES * a.shape[0],) + tuple(a.shape[1:])
                   for a in out_avals]
    zero_dtypes = [a.dtype for a in out_avals]
    zs = NamedSharding(mesh, PartitionSpec("core"))
    make_zeros = jax.jit(
        lambda: tuple(jnp.zeros(s, d)
                      for s, d in zip(zero_shapes, zero_dtypes)),
        out_shardings=(zs,) * n_outs)

    state = {"donate": None}

    def run(np_inputs: dict):
        donate_bufs = state["donate"]
        if donate_bufs is None:
            donate_bufs = make_zeros()
        args = [np_inputs[nm] for nm in in_names] + list(donate_bufs)
        outs = sharded(*args)
        # next call reuses these buffers as (donated) outputs; the kernel
        # overwrites every element so they need not be zero.
        state["donate"] = outs
        for o in outs:
            o.copy_to_host_async()
        return {nm: np.asarray(outs[i]) for i, nm in enumerate(out_names)}

    return run


def postprocess(y_g, d_g, sc_g, cfg: Cfg):
    B, C, PP, WD, NH = cfg.B, cfg.C, cfg.WD and cfg.PP, cfg.WD, cfg.NH
    B, C, PP, WD, NH = cfg.B, cfg.C, cfg.PP, cfg.WD, cfg.NH
    NCORES = cfg.NCORES
    NCr = PP * WD
    NNM = NCr // cfg.num_chunk
    HH = NCORES * PP
    y = y_g.reshape(NCORES, B, C, NNM, cfg.num_chunk)
    d = d_g.reshape(NCORES, B, NH, 1, NCr)
    sc = sc_g.reshape(NCORES, B, 128, 2, NNM)
    out = np.empty((B, C, HH, WD), np.float32)
    ov = out.reshape(B, NH, cfg.DV, NCORES, PP, WD)

    def work(c):
        # scale[b, p, t, ch] -> channel (t*128+p)
        s = np.transpose(sc[c], (0, 2, 1, 3)).reshape(B, C, NNM, 1)
        yc = y[c].astype(np.float32)
        yc *= s
        yc = yc.reshape(B, NH, cfg.DV, NCr)
        yc *= 1.0 / (d[c] * (1.0 + EPS))
        ov[:, :, :, c] = yc.reshape(B, NH, cfg.DV, PP, WD)
    list(_POOL.map(work, range(NCORES)))
    return out.reshape(B, C, HH, cfg.W, cfg.D)


def kernel(x, Wq, Wk, Wv, bv):
    cfg = Cfg()
    if "runner" not in _STATE:
        nc = build_nc(cfg)
        _STATE["runner"] = build_runner(nc, cfg)
    x = np.asarray(x, np.float32)
    inputs = prep_inputs(x, np.asarray(Wq, np.float32),
                         np.asarray(Wk, np.float32),
                         np.asarray(Wv, np.float32).reshape(cfg.C, 27),
                         np.asarray(bv, np.float32), cfg)
    outs = _STATE["runner"](inputs)
    return postprocess(outs["y"], outs["dnm"], outs["ysc"], cfg)


atexit.register(_POOL.shutdown, wait=False)


# revision 11
# speedup vs baseline: 7.0987x; 1.1726x over previous
"""nn_LinearConvAttention Trainium2 Bass kernel.

B=2, C=256, 48^3 grid, 4 heads (dqk=32, dv=64). 8 NeuronCores.

Sharding: 8-way spatial over H. Core c computes output planes [6c, 6c+6) of
both batch elements, all 256 channels. Inputs are uploaded as bf16 shards
[2, 256, 8, 48*48] (6 interior planes + 1 halo plane each side, zero-padded at
the global boundary). The only cross-core communication is a 131KB AllReduce
of the per-head kv/ksum statistics (kv contracts over the full spatial axis).

Math (per batch b):
  q = Wq x ; k = Wk x ; v = dwconv3x3x3(x) + bv
  ke = exp(k)                      (k rows are O(0.3); exp safe unshifted)
  kv[r, c] = sum_n ke[r, n] v[c, n]   ;  S[r] = sum_n ke[r, n]   (AllReduce)
  A = blockdiag_mask * kv / S[:, None]
  eq = exp(q)
  y[c, n] = sum_r A[r, c] eq[r, n]
  D[h, n] = sum_{r in h} eq[r, n]
  out = y / (D * (1 + eps))        (division folded into the host upcast;
                                    the reference's Z term == 1 exactly)
"""

import atexit
import contextlib
from concurrent.futures import ThreadPoolExecutor
from dataclasses import dataclass

import numpy as np
import ml_dtypes

import concourse.bacc as bacc
import concourse.mybir as mybir
from concourse.tile import TileContext

BF16 = mybir.dt.bfloat16
F32 = mybir.dt.float32
FP8 = mybir.dt.float8e3
I32 = mybir.dt.int32
EPS = 1e-6


@dataclass
class Cfg:
    B: int = 2
    C: int = 256
    NH: int = 4
    DQK: int = 32
    DV: int = 64
    W: int = 48
    D: int = 48
    PP: int = 6           # output planes per core
    NCORES: int = 8
    pe_ct: tuple = (True, False)   # conv unit engine per ctile: PE / DVE
    qk_chunk: int = 384
    num_chunk: int = 512

    @property
    def WD(self):
        return self.W * self.D

    @property
    def NC(self):
        return self.PP * self.WD

    @property
    def PIN(self):
        return self.PP + 2

    @property
    def DP(self):
        return self.D + 4   # padded D pitch (interior at col offset 2)

    @property
    def WDP(self):
        return (self.W + 2) * self.DP


def _tapidx(di, dj, dk):
    return (di + 1) * 9 + (dj + 1) * 3 + (dk + 1)


def build_nc(cfg: Cfg):
    WD, PIN, NC = cfg.WD, cfg.PIN, cfg.NC
    assert WD % 128 == 0

    nc = bacc.Bacc("TRN2", target_bir_lowering=False, debug=False,
                   num_devices=cfg.NCORES)

    x_in = nc.dram_tensor("x", [cfg.B, cfg.C, PIN, WD], FP8,
                          kind="ExternalInput").ap()
    wqk = nc.dram_tensor("wqk", [2, 128, 256], BF16, kind="ExternalInput").ap()
    wv = nc.dram_tensor("wv", [2, 27, 128], F32, kind="ExternalInput").ap()
    bv = nc.dram_tensor("bv", [2, 128], F32, kind="ExternalInput").ap()
    basis4 = nc.dram_tensor("basis4", [128, cfg.NH], BF16,
                            kind="ExternalInput").ap()
    bmask = nc.dram_tensor("bmask", [128, 256], BF16,
                           kind="ExternalInput").ap()
    y_out = nc.dram_tensor("y", [cfg.B, cfg.C, NC], mybir.dt.int8,
                           kind="ExternalOutput").ap()
    sc_out = nc.dram_tensor("ysc", [cfg.B, 128, 2, NC // cfg.num_chunk], F32,
                            kind="ExternalOutput").ap()
    d_out = nc.dram_tensor("dnm", [cfg.B, cfg.NH, NC], F32,
                           kind="ExternalOutput").ap()
    cc_in = nc.dram_tensor("cc_in", [cfg.B, 128, 257], F32)
    drm_dram = nc.dram_tensor("drm_b", [4, 1], F32)
    cc_out = nc.dram_tensor("cc_out", [cfg.B, 128, 257], F32)

    with TileContext(nc) as tc:
        _emit(nc, tc, cfg, x_in, wqk, wv, bv, basis4, bmask, y_out, d_out,
              cc_in, cc_out, sc_out, drm_dram)
    nc.compile()
    return nc


def _clip(s, n):
    """shift s in {-1,0,1}: returns (out_start, in_start, count)."""
    if s < 0:
        return 1, 0, n - 1
    if s > 0:
        return 0, 1, n - 1
    return 0, 0, n


def _emit(nc, tc, cfg, x_in, wqk, wv, bv, basis4, bmask, y_out, d_out,
          cc_in, cc_out, sc_out, drm_dram):
    WD, PP, PIN, NC, W, D = cfg.WD, cfg.PP, cfg.PIN, cfg.NC, cfg.W, cfg.D
    NCH = WD // 128
    QKC = cfg.qk_chunk
    assert WD % QKC == 0
    NQK = WD // QKC
    NMC = cfg.num_chunk
    assert NC % NMC == 0
    NNM = NC // NMC
    TAPS = [(di, dj, dk) for di in (-1, 0, 1) for dj in (-1, 0, 1)
            for dk in (-1, 0, 1)]
    TAPS.remove((0, 0, 0))
    TAPS.insert(0, (0, 0, 0))  # center first: defines psum/acc init

    ctx = contextlib.ExitStack()
    with ctx:
        const_p = ctx.enter_context(tc.tile_pool(name="const", bufs=1))
        xdv_p = ctx.enter_context(tc.tile_pool(name="xdv", bufs=4))
        x8_p = ctx.enter_context(tc.tile_pool(name="x8", bufs=3))
        xod_p = ctx.enter_context(tc.tile_pool(name="xod", bufs=4))
        eq_p = ctx.enter_context(tc.tile_pool(name="eq", bufs=2))
        ke_p = ctx.enter_context(tc.tile_pool(name="ke", bufs=2))
        v_p = ctx.enter_context(tc.tile_pool(name="v", bufs=3))
        keT_p = ctx.enter_context(tc.tile_pool(name="keT", bufs=2))
        vT_p = ctx.enter_context(tc.tile_pool(name="vT", bufs=2))
        out_p = ctx.enter_context(tc.tile_pool(name="out", bufs=3))
        small_p = ctx.enter_context(tc.tile_pool(name="small", bufs=2))
        akv_p = ctx.enter_context(tc.tile_pool(name="akv", bufs=2))

        qk_ps = ctx.enter_context(tc.tile_pool(name="qkps", bufs=1, space="PSUM"))
        cv_ps = ctx.enter_context(tc.tile_pool(name="cvps", bufs=1, space="PSUM"))
        kv_ps = ctx.enter_context(tc.tile_pool(name="kvps", bufs=1, space="PSUM"))
        nm_ps = ctx.enter_context(tc.tile_pool(name="nmps", bufs=1, space="PSUM"))

        # ---- constants ----
        wqk_sb = const_p.tile([128, 2, 256], BF16, tag="wqk")
        nc.sync.dma_start(out=wqk_sb[:, :, :],
                          in_=wqk.rearrange("t c m -> c t m"))
        wv_sb = const_p.tile([128, 2, 27], F32, tag="wv")
        nc.sync.dma_start(out=wv_sb[:, :, :],
                          in_=wv.rearrange("t k c -> c t k"))
        bv_sb = const_p.tile([128, 2], F32, tag="bv")
        nc.sync.dma_start(out=bv_sb[:, :], in_=bv.rearrange("t c -> c t"))
        basis_sb = const_p.tile([128, cfg.NH], BF16, tag="basis")
        nc.sync.dma_start(out=basis_sb[:, :], in_=basis4[:, :])
        mask_sb = const_p.tile([128, 256], BF16, tag="bmask")
        nc.sync.dma_start(out=mask_sb[:, :], in_=bmask[:, :])
        ones_sb = const_p.tile([128, 1], BF16, tag="ones")
        nc.vector.memset(ones_sb[:, :], 1.0)

        # identity & per-tap diagonal weight matrices for the PE conv ctiles
        iot = const_p.tile([128, 128], I32, tag="iot")
        nc.gpsimd.iota(iot[:, :], pattern=[[1, 128]], base=0,
                       channel_multiplier=-1)
        ident = const_p.tile([128, 128], BF16, tag="ident")
        nc.vector.tensor_scalar(ident[:, :], iot[:, :], 0, None,
                                op0=mybir.AluOpType.is_equal)
        diags = {}
        for ct in range(2):
            if not cfg.pe_ct[ct]:
                continue
            dg = const_p.tile([128, 27, 128], BF16, tag=f"diag{ct}")
            for t in range(27):
                nc.vector.tensor_scalar(dg[:, t, :], ident[:, :],
                                        wv_sb[:, ct, t:t + 1], None,
                                        op0=mybir.AluOpType.mult)
            diags[ct] = dg

        # persistent padded-x ring for the PE conv ctiles (borders stay 0)
        DP, WDP = cfg.DP, cfg.WDP
        XPE_SLOTS = 5
        xpe_ring = {}
        for ct in range(2):
            if not cfg.pe_ct[ct]:
                continue
            for s in range(XPE_SLOTS):
                t = const_p.tile([128, WDP], FP8, tag=f"xpr{ct}_{s}")
                nc.vector.memset(t[:, :], 0.0)
                xpe_ring[(ct, s)] = t
        ring_ctr = {ct: 0 for ct in range(2)}

        kv_sb = {}
        eq_sl = {}

        # =============== main loop over batches ===============
        for b in range(cfg.B):
            eq_slab = eq_p.tile([128, NC], BF16, tag="eq")
            eq_sl[b] = eq_slab
            kv_tile = kv_ps.tile([128, 256], F32, tag="kv")
            kvS_tile = kv_ps.tile([128, 1], F32, tag="kvS")
            first_kv = [True]

            xs = {}
            xso = {}

            def load_plane(pl, b=b, xs=xs, xso=xso):
                for ct in range(2):
                    if (pl, ct) in xs:
                        continue
                    src_ap = x_in[b, ct * 128:(ct + 1) * 128, pl, :]
                    if cfg.pe_ct[ct]:
                        t = xpe_ring[(ct, ring_ctr[ct] % XPE_SLOTS)]
                        ring_ctr[ct] += 1
                        dst = t[:, :].rearrange("c (w d) -> c w d", d=DP)
                        nc.sync.dma_start(
                            out=dst[:, 1:W + 1, 2:D + 2],
                            in_=src_ap.rearrange("c (w d) -> c w d", d=D))
                        xs[(pl, ct)] = t
                    else:
                        st8 = x8_p.tile([128, WD], FP8, tag=f"x8{ct}")
                        nc.sync.dma_start(out=st8[:, :], in_=src_ap)
                        t = xdv_p.tile([128, WD], BF16, tag=f"x{ct}")
                        nc.scalar.copy(t[:, :], st8[:, :])
                        xs[(pl, ct)] = t
                        to = xod_p.tile([128, WD], BF16, tag=f"xo{ct}")
                        # to[:, j] = x[:, j+1]; last element garbage, never read
                        nc.scalar.copy(to[:, 0:WD - 1], st8[:, 1:WD])
                        xso[(pl, ct)] = to

            for pl in range(min(3, PIN)):
                load_plane(pl)

            for p in range(PP):
                if p + 3 < PIN:
                    load_plane(p + 3)

                # ---- q/k projections (input plane p+1) + exp ----
                ke_pl = ke_p.tile([128, WD], BF16, tag="ke")
                for ch in range(NQK):
                    qp = qk_ps.tile([128, QKC], F32, tag="qps")
                    kp = qk_ps.tile([128, QKC], F32, tag="kps")
                    rows_per_qk = QKC // D
                    for ct in range(2):
                        if cfg.pe_ct[ct]:
                            xv = xs[(p + 1, ct)][:, :].rearrange(
                                "c (w d) -> c w d", d=DP)
                            r0 = ch * rows_per_qk
                            rhs = xv[:, 1 + r0:1 + r0 + rows_per_qk, 2:D + 2]
                        else:
                            rhs = xs[(p + 1, ct)][:, ch * QKC:(ch + 1) * QKC]
                        nc.tensor.matmul(qp[:, :], wqk_sb[:, ct, 0:128], rhs,
                                         start=(ct == 0), stop=(ct == 1))
                        nc.tensor.matmul(kp[:, :], wqk_sb[:, ct, 128:256], rhs,
                                         start=(ct == 0), stop=(ct == 1))
                    nc.scalar.activation(
                        eq_slab[:, p * WD + ch * QKC:p * WD + (ch + 1) * QKC],
                        qp[:, :], mybir.ActivationFunctionType.Exp)
                    nc.scalar.activation(ke_pl[:, ch * QKC:(ch + 1) * QKC],
                                         kp[:, :],
                                         mybir.ActivationFunctionType.Exp)

                # ---- depthwise conv ----
                v_pl = {}
                for ct in range(2):
                    vt = v_p.tile([128, WD], BF16, tag=f"v{ct}")
                    v_pl[ct] = vt
                    if cfg.pe_ct[ct]:
                        _conv_pe(nc, cfg, cv_ps, vt, xs, diags[ct], bv_sb,
                                 ct, p, TAPS)
                    else:
                        _conv_dve(nc, cfg, vt, xs, xso, wv_sb, bv_sb,
                                  ct, p, TAPS)

                # ---- transposes + kv accumulation ----
                keT = keT_p.tile([128, NCH, 128], BF16, tag="keT")
                nc.sync.dma_start_transpose(keT[:, :, :], ke_pl[:, :])
                vT = vT_p.tile([128, NCH, 256], BF16, tag="vT")
                nc.sync.dma_start_transpose(vT[:, :, 0:128], v_pl[0][:, :])
                nc.scalar.dma_start_transpose(vT[:, :, 128:256], v_pl[1][:, :])
                for ch in range(NCH):
                    st = first_kv[0]
                    last = (p == PP - 1 and ch == NCH - 1)
                    nc.tensor.matmul(kv_tile[:, :], keT[:, ch, :],
                                     vT[:, ch, :], start=st, stop=last,
                                     skip_group_check=True)
                    nc.tensor.matmul(kvS_tile[:, :], keT[:, ch, :],
                                     ones_sb[:, :], start=st, stop=last,
                                     skip_group_check=True)
                    first_kv[0] = False

            # ---- AllReduce kv / S, then A = mask * kv / S ----
            kvs = akv_p.tile([128, 257], F32, tag="kvsb")
            nc.vector.tensor_copy(kvs[:, 0:256], kv_tile[:, :])
            nc.vector.tensor_copy(kvs[:, 256:257], kvS_tile[:, :])
            nc.sync.dma_start(out=cc_in.ap()[b, :, :], in_=kvs[:, :])
            nc.gpsimd.collective_compute(
                "AllReduce", mybir.AluOpType.add,
                replica_groups=[list(range(cfg.NCORES))],
                ins=[cc_in.ap()[b:b + 1, :, :].opt()],
                outs=[cc_out.ap()[b:b + 1, :, :].opt()])
            kvr = akv_p.tile([128, 257], F32, tag="kvr")
            nc.scalar.dma_start(out=kvr[:, :], in_=cc_out.ap()[b, :, :])
            rS = small_p.tile([128, 1], F32, tag="rS")
            nc.vector.reciprocal(rS[:, :], kvr[:, 256:257])
            A_sb = akv_p.tile([128, 256], BF16, tag="A")
            nc.vector.scalar_tensor_tensor(
                A_sb[:, :], kvr[:, 0:256], rS[:, 0:1], mask_sb[:, :],
                op0=mybir.AluOpType.mult, op1=mybir.AluOpType.mult)
            kv_sb[b] = A_sb

        # =============== output phase ===============
        for b in range(cfg.B):
            A_sb = kv_sb[b]
            eq_slab = eq_sl[b]
            # |y[c,n]| <= max_r |A[r,c]| * D[h(c),n]: per-channel |A| max via
            # xbar transpose (partition-aligned with the output tiles).
            AT = akv_p.tile([128, 2, 128], BF16, tag="AT")
            nc.sync.dma_start_transpose(AT[:, :, :], A_sb[:, :])
            maxA = small_p.tile([128, 2], F32, tag="maxA")
            for ct in range(2):
                nc.vector.tensor_reduce(
                    maxA[:, ct:ct + 1], AT[:, ct, :],
                    axis=mybir.AxisListType.X, op=mybir.AluOpType.max,
                    apply_absolute_value=True)
            # pre-divide by the int8 target amplitude
            maxAp = small_p.tile([128, 2], F32, tag="maxAp")
            nc.vector.tensor_scalar(maxAp[:, :], maxA[:, :], 1.0 / 126.0,
                                    None, op0=mybir.AluOpType.mult)
            sc_stage = out_p.tile([128, 2, NNM], F32, tag="scst")
            for ch in range(NNM):
                rhs = eq_slab[:, ch * NMC:(ch + 1) * NMC]
                n0 = nm_ps.tile([128, NMC], F32, tag="n0")
                n1 = nm_ps.tile([128, NMC], F32, tag="n1")
                dr = nm_ps.tile([cfg.NH, NMC], F32, tag="dr")
                nc.tensor.matmul(n0[:, :], A_sb[:, 0:128], rhs,
                                 start=True, stop=True)
                nc.tensor.matmul(n1[:, :], A_sb[:, 128:256], rhs,
                                 start=True, stop=True)
                nc.tensor.matmul(dr[:, :], basis_sb[:, :], rhs,
                                 start=True, stop=True)
                # per-head chunk max of D, broadcast to all 128 partitions
                # via a DRAM bounce with a stride-0 read
                drm = small_p.tile([cfg.NH, 1], F32, tag="drm")
                nc.vector.tensor_reduce(drm[:, :], dr[:, :],
                                        axis=mybir.AxisListType.X,
                                        op=mybir.AluOpType.max)
                nc.gpsimd.dma_start(out=drm_dram.ap()[:, :], in_=drm[:, :])
                drb = small_p.tile([128, 1], F32, tag="drb")
                nc.gpsimd.dma_start(
                    out=drb[:, :],
                    in_=drm_dram.ap().broadcast_to([cfg.NH, 128 // cfg.NH, 1]))
                for ct in range(2):
                    nc.vector.tensor_scalar(
                        sc_stage[:, ct, ch:ch + 1], drb[:, :],
                        maxAp[:, ct:ct + 1], None, op0=mybir.AluOpType.mult)
                    s = small_p.tile([128, 1], F32, tag=f"s{ct}")
                    nc.vector.reciprocal(s[:, :], sc_stage[:, ct, ch:ch + 1])
                    oi = out_p.tile([128, NMC], mybir.dt.int8, tag=f"oi{ct}")
                    nt = n0 if ct == 0 else n1
                    if ct == 0:
                        nc.vector.tensor_scalar(
                            oi[:, :], nt[:, :], s[:, 0:1], None,
                            op0=mybir.AluOpType.mult)
                    else:
                        nc.scalar.activation(
                            oi[:, :], nt[:, :],
                            mybir.ActivationFunctionType.Copy,
                            scale=s[:, 0:1])
                    nc.sync.dma_start(
                        out=y_out[b, ct * 128:(ct + 1) * 128,
                                  ch * NMC:(ch + 1) * NMC],
                        in_=oi[:, :])
                od = out_p.tile([cfg.NH, NMC], F32, tag="od")
                nc.vector.tensor_copy(od[:, :], dr[:, :])
                nc.gpsimd.dma_start(
                    out=d_out[b, :, ch * NMC:(ch + 1) * NMC], in_=od[:, :])
            nc.sync.dma_start(out=sc_out[b, :, :, :], in_=sc_stage[:, :, :])


def _conv_pe(nc, cfg, cv_ps, vt, xs, dg, bv_sb, ct, p, taps):
    """Conv unit on PE: per-tap diagonal-weight matmuls accumulating into
    PSUM pieces of the plane; input is the zero-padded plane tile so every
    tap is a uniform full-width window. Evicted via ACT with +bv bias."""
    W, D, DP = cfg.W, cfg.D, cfg.DP
    rows_per = max(1, 512 // D)
    n_pieces = (W + rows_per - 1) // rows_per
    for pc in range(n_pieces):
        t0, t1 = pc * rows_per, min(W, (pc + 1) * rows_per)
        nr = t1 - t0
        ps = cv_ps.tile([128, nr * D], F32, tag="cv")
        for i, (di, dj, dk) in enumerate(taps):
            xv = xs[(p + 1 + di, ct)][:, :].rearrange(
                "c (w d) -> c w d", d=DP)
            rhs = xv[:, t0 + dj + 1:t1 + dj + 1, 2 + dk:2 + dk + D]
            nc.tensor.matmul(
                ps[:, :], dg[:, _tapidx(di, dj, dk), :], rhs,
                start=(i == 0), stop=(i == len(taps) - 1),
                skip_group_check=True)
        nc.scalar.activation(
            vt[:, t0 * D:t1 * D], ps[:, :],
            mybir.ActivationFunctionType.Identity,
            bias=bv_sb[:, ct:ct + 1])


def _conv_dve(nc, cfg, vt, xs, xso, wv_sb, bv_sb, ct, p, taps):
    """Conv unit on DVE: scalar_tensor_tensor FMA into the bf16 v tile.
    D-axis (innermost) alignment for 2x mode:
      dk=0  : both APs 4B-aligned as-is
      dk=+1 : src from the odd-shifted copy xo (xo[:, j] = x[:, j+1])
      dk=-1 : dst cols [2, D) with src xo cols [0, D-2); col 1 fixed up
              with a small strided op (col 0 needs no contribution).
    """
    W, D = cfg.W, cfg.D

    def w_ap(tap):
        i = _tapidx(*tap)
        return wv_sb[:, ct, i:i + 1]

    for i, (di, dj, dk) in enumerate(taps):
        ow0, iw0, wcnt = _clip(dj, W)
        xt = xs[(p + 1 + di, ct)]
        ov = vt[:, :].rearrange("c (w d) -> c w d", d=D)
        if i == 0:
            nc.vector.tensor_scalar(
                vt[:, :], xt[:, :], w_ap((0, 0, 0)), bv_sb[:, ct:ct + 1],
                op0=mybir.AluOpType.mult, op1=mybir.AluOpType.add)
            continue
        if dk == 0:
            xv = xt[:, :].rearrange("c (w d) -> c w d", d=D)
            dst = ov[:, ow0:ow0 + wcnt, :]
            src = xv[:, iw0:iw0 + wcnt, :]
        elif dk == 1:
            xo = xso[(p + 1 + di, ct)][:, :].rearrange("c (w d) -> c w d", d=D)
            dst = ov[:, ow0:ow0 + wcnt, 0:D - 1]
            src = xo[:, iw0:iw0 + wcnt, 0:D - 1]
        else:  # dk == -1
            xo = xso[(p + 1 + di, ct)][:, :].rearrange("c (w d) -> c w d", d=D)
            dst = ov[:, ow0:ow0 + wcnt, 2:D]
            src = xo[:, iw0:iw0 + wcnt, 0:D - 2]
        nc.vector.scalar_tensor_tensor(
            dst, src, w_ap((di, dj, dk)), dst,
            op0=mybir.AluOpType.mult, op1=mybir.AluOpType.add)
        if dk == -1:
            xv = xt[:, :].rearrange("c (w d) -> c w d", d=D)
            d1 = ov[:, ow0:ow0 + wcnt, 1:2]
            s0 = xv[:, iw0:iw0 + wcnt, 0:1]
            nc.vector.scalar_tensor_tensor(
                d1, s0, w_ap((di, dj, dk)), d1,
                op0=mybir.AluOpType.mult, op1=mybir.AluOpType.add)


# ======================================================================
# host side
# ======================================================================

_STATE = {}
_POOL = ThreadPoolExecutor(4)


def _make_shard_x(x, c, cfg: Cfg):
    """Per-core fp8 x shard [B, C, PIN, WD] with halo planes (zero at edges)."""
    B, C, PP, PIN, WD = cfg.B, cfg.C, cfg.PP, cfg.PIN, cfg.WD
    HH = cfg.NCORES * PP
    xr = x.reshape(B, C, HH, WD)
    sh = np.zeros((B, C, PIN, WD), ml_dtypes.float8_e3m4)
    lo = c * PP - 1
    hi = c * PP + PP + 1
    slo = max(lo, 0)
    shi = min(hi, HH)
    sh[:, :, slo - lo:shi - lo] = xr[:, :, slo:shi]
    return sh


def _make_weights(Wq, Wk, Wv27, bvec, cfg: Cfg):
    wqk1 = np.empty((2, 128, 256), ml_dtypes.bfloat16)
    for ct in range(2):
        wqk1[ct, :, 0:128] = Wq[:, ct * 128:(ct + 1) * 128].T
        wqk1[ct, :, 128:256] = Wk[:, ct * 128:(ct + 1) * 128].T
    wv1 = np.empty((2, 27, 128), np.float32)
    for ct in range(2):
        wv1[ct] = Wv27[ct * 128:(ct + 1) * 128].T
    bv1 = bvec.reshape(2, 128).astype(np.float32)
    basis = np.zeros((128, cfg.NH), ml_dtypes.bfloat16)
    for r in range(128):
        basis[r, r // cfg.DQK] = 1
    bm = np.zeros((128, 256), ml_dtypes.bfloat16)
    for r in range(128):
        h = r // cfg.DQK
        bm[r, h * cfg.DV:(h + 1) * cfg.DV] = 1
    return {"wqk": wqk1, "wv": wv1, "bv": bv1, "basis4": basis, "bmask": bm}


def prep_inputs(x, Wq, Wk, Wv27, bvec, cfg: Cfg):
    """Non-pipelined variant (used by dev_test); returns concat np arrays."""
    ws = _make_weights(Wq, Wk, Wv27, bvec, cfg)
    shards = [_make_shard_x(x, c, cfg) for c in range(cfg.NCORES)]
    out = {"x": np.concatenate(shards, axis=0)}
    for k, v in ws.items():
        out[k] = np.concatenate([v] * cfg.NCORES, axis=0).reshape(
            (cfg.NCORES * v.shape[0],) + v.shape[1:])
    return out


def build_runner(nc, cfg: Cfg):
    """Pipelined runner: per-core shard production overlaps async per-device
    puts; per-shard fetches overlap per-core postprocessing. Donated output
    buffers are fed back from the previous call (kernel writes every element).
    """
    import jax
    import jax.numpy as jnp
    from jax.experimental.shard_map import shard_map
    from jax.sharding import Mesh, PartitionSpec, NamedSharding
    from concourse import bass2jax

    bass2jax.install_neuronx_cc_hook()

    partition_name = (nc.partition_id_tensor.name
                      if nc.partition_id_tensor else None)
    in_names, out_names, out_avals = [], [], []
    for alloc in nc.m.functions[0].allocations:
        if not isinstance(alloc, mybir.MemoryLocationSet):
            continue
        name = alloc.memorylocations[0].name
        if alloc.kind == "ExternalInput":
            if name != partition_name:
                in_names.append(name)
        elif alloc.kind == "ExternalOutput":
            out_names.append(name)
            out_avals.append(jax.core.ShapedArray(
                tuple(alloc.tensor_shape), mybir.dt.np(alloc.dtype)))
    n_params = len(in_names)
    n_outs = len(out_names)
    all_names = in_names + out_names
    if partition_name is not None:
        all_names = all_names + [partition_name]
    donate = tuple(range(n_params, n_params + n_outs))

    def _body(*args):
        operands = list(args)
        if partition_name is not None:
            operands.append(bass2jax.partition_id_tensor())
        outs = bass2jax._bass_exec_p.bind(
            *operands,
            out_avals=tuple(out_avals),
            in_names=tuple(all_names),
            out_names=tuple(out_names),
            lowering_input_output_aliases=(),
            sim_require_finite=True,
            sim_require_nnan=True,
            nc=nc,
        )
        return tuple(outs)

    devices = jax.devices()[:cfg.NCORES]
    mesh = Mesh(np.asarray(devices), ("core",))
    in_specs = (PartitionSpec("core"),) * (n_params + n_outs)
    out_specs = (PartitionSpec("core"),) * n_outs
    sharding = NamedSharding(mesh, PartitionSpec("core"))
    sharded = jax.jit(
        shard_map(_body, mesh=mesh, in_specs=in_specs, out_specs=out_specs,
                  check_rep=False),
        donate_argnums=donate, keep_unused=True)

    zero_shapes = [(cfg.NCORES * a.shape[0],) + tuple(a.shape[1:])
                   for a in out_avals]
    zero_dtypes = [a.dtype for a in out_avals]
    make_zeros = jax.jit(
        lambda: tuple(jnp.zeros(s, d)
                      for s, d in zip(zero_shapes, zero_dtypes)),
        out_shardings=(sharding,) * n_outs)

    state = {"donate": None, "wcache": None}

    def run(shard_fn, weight_arrs):
        # weights: put each (tiny) replicated array once per call unless cached
        wkey = tuple(id(weight_arrs[nm]) for nm in in_names if nm != "x")
        if state["wcache"] is not None and state["wcache"][0] == wkey:
            wglob = state["wcache"][1]
        else:
            wglob = {}
            for nm in in_names:
                if nm == "x":
                    continue
                w = weight_arrs[nm]
                shards = [jax.device_put(w, devices[c])
                          for c in range(cfg.NCORES)]
                wglob[nm] = jax.make_array_from_single_device_arrays(
                    (cfg.NCORES * w.shape[0],) + w.shape[1:], sharding, shards)
            state["wcache"] = (wkey, wglob)

        # x: produce shard c on CPU while shard c-1 streams to its device
        xshards = []
        shard_shape = None
        for c in range(cfg.NCORES):
            s = shard_fn(c)
            shard_shape = s.shape
            xshards.append(jax.device_put(s, devices[c]))
        xglob = jax.make_array_from_single_device_arrays(
            (cfg.NCORES * shard_shape[0],) + shard_shape[1:], sharding,
            xshards)

        donate_bufs = state["donate"]
        if donate_bufs is None:
            donate_bufs = make_zeros()
        args = []
        for nm in in_names:
            args.append(xglob if nm == "x" else wglob[nm])
        args.extend(donate_bufs)
        outs = sharded(*args)
        state["donate"] = outs
        for o in outs:
            o.copy_to_host_async()
        return {nm: outs[i] for i, nm in enumerate(out_names)}, out_names

    return run


def postprocess(y_g, d_g, sc_g, cfg: Cfg):
    """np-array variant used by dev_test."""
    B, C, PP, WD, NH = cfg.B, cfg.C, cfg.PP, cfg.WD, cfg.NH
    NCORES = cfg.NCORES
    NCr = PP * WD
    NNM = NCr // cfg.num_chunk
    HH = NCORES * PP
    out = np.empty((B, C, HH, WD), np.float32)
    ov = out.reshape(B, NH, cfg.DV, NCORES, PP, WD)
    y = y_g.reshape(NCORES, B, C, NNM, cfg.num_chunk)
    d = d_g.reshape(NCORES, B, NH, 1, NCr)
    sc = sc_g.reshape(NCORES, B, 128, 2, NNM)
    for c in range(NCORES):
        _post_core(ov, y[c], d[c], sc[c], c, cfg)
    return out.reshape(B, C, HH, cfg.W, cfg.D)


def _post_core(ov, yc_i8, dc, scc, c, cfg: Cfg):
    B, C, NH = cfg.B, cfg.C, cfg.NH
    NCr = cfg.PP * cfg.WD
    NNM = NCr // cfg.num_chunk
    s = np.transpose(scc.reshape(B, 128, 2, NNM),
                     (0, 2, 1, 3)).reshape(B, C, NNM, 1)
    yc = yc_i8.reshape(B, C, NNM, cfg.num_chunk).astype(np.float32)
    yc *= s
    yc = yc.reshape(B, NH, cfg.DV, NCr)
    yc *= 1.0 / (dc.reshape(B, NH, 1, NCr) * (1.0 + EPS))
    ov[:, :, :, c] = yc.reshape(B, NH, cfg.DV, cfg.PP, cfg.WD)


def kernel(x, Wq, Wk, Wv, bv):
    cfg = Cfg()
    if "runner" not in _STATE:
        nc = build_nc(cfg)
        _STATE["runner"] = build_runner(nc, cfg)
    x = np.asarray(x, np.float32)
    ws = _make_weights(np.asarray(Wq, np.float32), np.asarray(Wk, np.float32),
                       np.asarray(Wv, np.float32).reshape(cfg.C, 27),
                       np.asarray(bv, np.float32), cfg)

    outs, _ = _STATE["runner"](lambda c: _make_shard_x(x, c, cfg), ws)

    # per-shard fetch overlapped with per-core postprocess
    B, C, PP, WD = cfg.B, cfg.C, cfg.PP, cfg.WD
    HH = cfg.NCORES * PP
    out = np.empty((B, C, HH, WD), np.float32)
    ov = out.reshape(B, cfg.NH, cfg.DV, cfg.NCORES, PP, WD)
    def _shards(a):
        return sorted(a.addressable_shards,
                      key=lambda s: (s.index[0].start or 0))
    y_sh = _shards(outs["y"])
    d_sh = _shards(outs["dnm"])
    s_sh = _shards(outs["ysc"])
    for c in range(cfg.NCORES):
        yc = np.asarray(y_sh[c].data)
        dc = np.asarray(d_sh[c].data)
        scc = np.asarray(s_sh[c].data)
        _post_core(ov, yc, dc, scc, c, cfg)
    return out.reshape(B, C, HH, cfg.W, cfg.D)


atexit.register(_POOL.shutdown, wait=False)
